# revision 13
# baseline (speedup 1.0000x reference)
"""Two-layer single-head GAT on Trainium2 (8 NeuronCores, Bass/Tile).

Strategy (graph-parallel over dst nodes), v2:
  - Nodes relabeled into slots: 8 cores x NW windows x 64 slots. Nodes
    assigned to cores balanced by degree (serpentine over degree-sorted
    order), then packed into windows (<=64 nodes, per-src-bucket in-degree
    <= 256; bucket(src) = src_core//2).
  - Per layer each core builds its shard of a node table
    row[n] = [h(64) | 1.0 | as_hi | as_lo] bf16 (128-col rows for the
    256B-stride dma_gather), AllGathered so every core holds the full
    table in DRAM.
  - Edges live on the dst core, grouped by (window-group g of GRP=8
    windows, src bucket b). Per (g,b) one dma_gather of CALL=2048 rows
    (int16 idx within the bucket slice), issued on SWDGE queue b so the
    four buckets' Q7 descriptor generation overlaps across core pairs.
  - Per (g,b), batched DVE ops build M = onehot(dstl) [128e x 16t x 64d],
    Madc = M * adrep (scratch), adpec = rowsum(Madc) = a_d[dst_e]; scores
    s = as_hi+as_lo+adpec, ex = exp(lrelu(s)) (ACT); Mex = M * ex.
  - Aggregation per window: PSUM [64d x 65] += Mex_tile^T @ vslab_tile
    (lhsT = Mex [128e x 64d], rhs = raw gathered rows [128e x 65] =
    [h | 1]); epilogue divides by the ex-sum column (col 64), adds bias;
    layer 1 transposes back to [64f x 64s] via identity matmul + ACT relu
    copy into the resident x2T; layer 2 DMAs slot-major fp32 output.
Softmax max-subtraction is skipped (scores are O(10); exp stays in fp32
range; alpha is mathematically identical). Output unpermuted on host.
"""

import numpy as np
import ml_dtypes

BF16 = ml_dtypes.bfloat16

NCORES = 8
P = 128
D = 64
NEG_SLOPE = 0.2
EPS = 1e-16

WSLOT = 64        # dst slots per window
NODECAP = 64      # max nodes per window
TPB = 2           # 128-edge tiles per (window, bucket)
BCAP = TPB * 128  # per-bucket in-degree cap per window
GRP = 8           # windows per gather group
GT = GRP * TPB    # tiles per (group, bucket)
CALL = GT * 128   # idxs per dma_gather call (keep < ~2500: walrus 16-bit)
FILL = 999.0      # dstl filler (never matches iota 0..63)

_CACHE = {}


def _preprocess(x, edge_index):
    """Host-side partitioning/indexing. Returns per-core input arrays + meta."""
    N = x.shape[0]
    src = edge_index[0].astype(np.int64)
    dst = edge_index[1].astype(np.int64)

    deg = np.bincount(dst, minlength=N)

    # --- assign nodes to cores: serpentine over degree-sorted order ---
    order = np.argsort(-deg, kind="stable")
    core_of_node = np.empty(N, np.int32)
    pat = np.concatenate([np.arange(NCORES), np.arange(NCORES)[::-1]])
    core_of_node[order] = pat[np.arange(N) % (2 * NCORES)]

    bucket_of_node = core_of_node // 2  # 4 buckets of 2 cores each

    deg_b = np.zeros((N, 4), np.int64)
    for b in range(4):
        m = bucket_of_node[src] == b
        deg_b[:, b] = np.bincount(dst[m], minlength=N)

    # --- pack windows per core: <=NODECAP nodes, per-bucket deg <= BCAP ---
    windows = [[] for _ in range(NCORES)]
    for c in range(NCORES):
        nodes_c = order[core_of_node[order] == c]  # degree-sorted
        cur, cur_b = [], np.zeros(4, np.int64)
        for n in nodes_c:
            db = deg_b[n]
            if cur and (len(cur) >= NODECAP or np.any(cur_b + db > BCAP)):
                windows[c].append(cur)
                cur, cur_b = [], np.zeros(4, np.int64)
            cur.append(n)
            cur_b = cur_b + db
        if cur:
            windows[c].append(cur)

    nw_real = max(len(w) for w in windows)
    NG = -(-nw_real // GRP)
    NW = NG * GRP
    SLOTS_PC = NW * WSLOT
    NSLOT = NCORES * SLOTS_PC
    BSZ = NSLOT // 4
    assert BSZ <= 32767, f"int16 gather range exceeded: BSZ={BSZ}"
    NBLK = SLOTS_PC // P  # phase-A blocks of 128 slots

    # --- slot assignment ---
    slot_of_node = np.full(N, -1, np.int64)
    for c in range(NCORES):
        for w, wl in enumerate(windows[c]):
            base = c * SLOTS_PC + w * WSLOT
            slot_of_node[np.asarray(wl, np.int64)] = base + np.arange(len(wl))
    assert (slot_of_node >= 0).all()

    sslot = slot_of_node[src]
    dslot = slot_of_node[dst]
    ecore = (dslot // SLOTS_PC).astype(np.int64)
    ew = (dslot % SLOTS_PC) // WSLOT
    eb = (sslot // BSZ).astype(np.int64)
    edloc = (dslot % WSLOT).astype(np.int64)
    esidx = (sslot % BSZ).astype(np.int64)

    key = ((ecore * NW + ew) * 4 + eb)
    eorder = np.argsort(key, kind="stable")
    key_s = key[eorder]
    cnt = np.bincount(key_s, minlength=NCORES * NW * 4)
    assert cnt.max() <= BCAP, f"bucket cap violated: {cnt.max()}"
    starts = np.zeros(NCORES * NW * 4 + 1, np.int64)
    np.cumsum(cnt, out=starts[1:])
    esidx_s = esidx[eorder]
    edloc_s = edloc[eorder]

    # gidx: [C, NG, 4, CALL]; dloc: [C, NG, 4, GT, 128]
    gidx = np.zeros((NCORES, NG, 4, CALL), np.int16)
    dloc = np.full((NCORES, NG, 4, GT, P), FILL, np.float32)
    for c in range(NCORES):
        for w in range(NW):
            g, wl = divmod(w, GRP)
            for b in range(4):
                k = (c * NW + w) * 4 + b
                lo, hi = starts[k], starts[k + 1]
                n = hi - lo
                if n == 0:
                    continue
                off = wl * TPB * P
                gidx[c, g, b, off:off + n] = esidx_s[lo:hi].astype(np.int16)
                tt = (np.arange(n) // P) + wl * TPB
                pp = np.arange(n) % P
                dloc[c, g, b, tt, pp] = edloc_s[lo:hi].astype(np.float32)

    # wrap-16 + replicate to 128 partitions: [C, NG*4*128, CALL//16]
    g16 = gidx.reshape(NCORES, NG * 4, CALL // 16, 16).transpose(0, 1, 3, 2)
    g128 = np.tile(g16, (1, 1, 8, 1)).reshape(NCORES, NG * 4 * 128, CALL // 16)
    # dstl: [C, 128, NG * 64]  col = g*64 + b*GT + t
    dl = dloc.transpose(0, 4, 1, 2, 3).reshape(NCORES, P, NG * 4 * GT)
    dl = np.ascontiguousarray(dl).astype(BF16)

    node_of_slot = np.full(NSLOT, -1, np.int64)
    node_of_slot[slot_of_node] = np.arange(N)
    xT = np.zeros((NCORES, D, SLOTS_PC), np.float32)
    for c in range(NCORES):
        sl = node_of_slot[c * SLOTS_PC:(c + 1) * SLOTS_PC]
        valid = sl >= 0
        blk = np.zeros((SLOTS_PC, D), np.float32)
        blk[valid] = x[sl[valid]]
        xT[c] = blk.T

    meta = dict(NW=NW, NG=NG, NBLK=NBLK, SLOTS_PC=SLOTS_PC, NSLOT=NSLOT,
                BSZ=BSZ, N=N)
    percore = dict(xT=xT, gidx=g128, dstl=dl)
    return meta, percore, node_of_slot


def _build_program(meta):
    import concourse.bacc as bacc
    import concourse.tile as tile
    from concourse import mybir

    F32, BF, I16 = mybir.dt.float32, mybir.dt.bfloat16, mybir.dt.int16
    Alu = mybir.AluOpType
    Act = mybir.ActivationFunctionType
    AxL = mybir.AxisListType

    NW, NG, NBLK = meta["NW"], meta["NG"], meta["NBLK"]
    SLOTS_PC, NSLOT, BSZ = meta["SLOTS_PC"], meta["NSLOT"], meta["BSZ"]
    BPG = GRP * WSLOT // P  # phase-A blocks per window-group (4)

    nc = bacc.Bacc("TRN2", target_bir_lowering=False, debug=False,
                   num_devices=NCORES, num_swdge_queues=4)

    xT_d = nc.dram_tensor("xT", [D, SLOTS_PC], F32, kind="ExternalInput")
    gidx_d = nc.dram_tensor("gidx", [NG * 4 * 128, CALL // 16], I16,
                            kind="ExternalInput")
    dstl_d = nc.dram_tensor("dstl", [P, NG * 4 * GT], BF,
                            kind="ExternalInput")
    w1cat_d = nc.dram_tensor("w1cat", [D, 67], F32, kind="ExternalInput")
    w2cat_d = nc.dram_tensor("w2cat", [D, 67], BF, kind="ExternalInput")
    wd1_d = nc.dram_tensor("wd1rep", [D, 128], F32, kind="ExternalInput")
    wd2_d = nc.dram_tensor("wd2rep", [D, 128], BF, kind="ExternalInput")
    b1rep_d = nc.dram_tensor("b1rep", [WSLOT, D], BF, kind="ExternalInput")
    b2rep_d = nc.dram_tensor("b2rep", [WSLOT, D], F32, kind="ExternalInput")
    ident_d = nc.dram_tensor("ident", [WSLOT, WSLOT], BF, kind="ExternalInput")
    out_d = nc.dram_tensor("outS", [SLOTS_PC, D], F32, kind="ExternalOutput")

    shard = [nc.dram_tensor(f"shard{l}", [SLOTS_PC, 128], BF) for l in (1, 2)]
    tbl = [nc.dram_tensor(f"tbl{l}", [NSLOT, 128], BF, addr_space="Shared")
           for l in (1, 2)]

    with tile.TileContext(nc) as tc:
        import contextlib
        stack = contextlib.ExitStack()
        with stack:
            const = stack.enter_context(tc.tile_pool(name="const", bufs=1))
            small = stack.enter_context(tc.tile_pool(name="small", bufs=3))
            vp = stack.enter_context(tc.tile_pool(name="vp", bufs=3))
            mp = stack.enter_context(tc.tile_pool(name="mp", bufs=3))
            sc = stack.enter_context(tc.tile_pool(name="sc", bufs=2))
            ep = stack.enter_context(tc.tile_pool(name="ep", bufs=2))
            psA = stack.enter_context(tc.tile_pool(name="psA", bufs=2, space="PSUM"))
            psB = stack.enter_context(tc.tile_pool(name="psB", bufs=1, space="PSUM"))
            psC = stack.enter_context(tc.tile_pool(name="psC", bufs=1, space="PSUM"))

            # constants
            iota_i = const.tile([P, WSLOT], I16)
            nc.gpsimd.iota(iota_i[:], pattern=[[1, WSLOT]], base=0,
                           channel_multiplier=0)
            iota_b = const.tile([P, WSLOT], BF)
            nc.vector.tensor_copy(iota_b[:], iota_i[:])
            w1cat = const.tile([D, 67], F32)
            nc.sync.dma_start(w1cat[:], w1cat_d.ap()[:])
            w2cat = const.tile([D, 67], BF)
            nc.sync.dma_start(w2cat[:], w2cat_d.ap()[:])
            wd1 = const.tile([D, 128], F32)
            nc.sync.dma_start(wd1[:], wd1_d.ap()[:])
            wd2 = const.tile([D, 128], BF)
            nc.sync.dma_start(wd2[:], wd2_d.ap()[:])
            b1rep = const.tile([WSLOT, D], BF)
            nc.sync.dma_start(b1rep[:], b1rep_d.ap()[:])
            b2rep = const.tile([WSLOT, D], F32)
            nc.sync.dma_start(b2rep[:], b2rep_d.ap()[:])
            ident = const.tile([WSLOT, WSLOT], BF)
            nc.sync.dma_start(ident[:], ident_d.ap()[:])

            # resident across layers
            x2T = const.tile([D, SLOTS_PC], BF)
            adrep = const.tile([P, SLOTS_PC], BF)

            def phase_a_block(layer, blk):
                wcat = w1cat if layer == 0 else w2cat
                wdrep = wd1 if layer == 0 else wd2
                shard_l = shard[layer].ap()
                if layer == 0:
                    xtw = small.tile([D, P], F32, tag="xtw")
                    nc.sync.dma_start(xtw[:], xT_d.ap()[:, blk * P:(blk + 1) * P])
                    lhs = xtw[:]
                else:
                    lhs = x2T[:, blk * P:(blk + 1) * P]
                ps_c = psA.tile([P, 67], F32)
                nc.tensor.matmul(ps_c[:], lhsT=lhs, rhs=wcat[:],
                                 start=True, stop=True)
                ps_ad = psB.tile([P, 128], F32)
                nc.tensor.matmul(ps_ad[:], lhsT=wdrep[:], rhs=lhs,
                                 start=True, stop=True)
                tblrow = small.tile([P, 67], BF, tag="tblrow")
                nc.scalar.copy(tblrow[:], ps_c[:])
                nc.vector.memset(tblrow[:, 64:65], 1.0)
                # a_s_lo residual for extra precision
                nc.vector.tensor_tensor(out=tblrow[:, 66:67],
                                        in0=ps_c[:, 65:66],
                                        in1=tblrow[:, 65:66],
                                        op=Alu.subtract)
                nc.sync.dma_start(shard_l[blk * P:(blk + 1) * P, 0:67],
                                  tblrow[:])
                nc.vector.tensor_copy(adrep[:, blk * P:(blk + 1) * P],
                                      ps_ad[:])

            def all_gather(layer):
                nc.gpsimd.collective_compute(
                    "AllGather", mybir.AluOpType.bypass,
                    replica_groups=[list(range(NCORES))],
                    ins=[shard[layer].ap()[:, :]], outs=[tbl[layer].ap()[:, :]],
                )

            def edge_group(layer, g):
                tbl_l = tbl[layer].ap()
                dstl = sc.tile([P, 4 * GT], BF, tag="dstl")
                nc.sync.dma_start(
                    dstl[:],
                    dstl_d.ap()[:, g * 4 * GT:(g + 1) * 4 * GT])
                vslab = vp.tile([P, 4, GT, 128], BF, tag="vslab")
                for b in range(4):
                    idxt = small.tile([128, CALL // 16], I16, tag="idxt")
                    r0 = (g * 4 + b) * 128
                    nc.sync.dma_start(idxt[:], gidx_d.ap()[r0:r0 + 128, :])
                    nc.gpsimd.dma_gather(
                        out_ap=vslab[:, b, :, :],
                        in_ap=tbl_l[b * BSZ:(b + 1) * BSZ, :],
                        idxs_ap=idxt[:], num_idxs=CALL, num_idxs_reg=CALL,
                        elem_size=128, single_packet=False, queue_num=b)

                M = mp.tile([P, 4, GT, WSLOT], BF, tag="M")
                madc = sc.tile([P, GT, WSLOT], BF, tag="madc")
                adpec = sc.tile([P, 4 * GT], F32, tag="adpec")
                nc.vector.tensor_tensor(
                    out=M[:],
                    in0=iota_b[:, None, None, :]
                        .to_broadcast([P, 4, GT, WSLOT]),
                    in1=dstl[:].rearrange("p (b t) -> p b t", b=4)
                        [:, :, :, None].to_broadcast([P, 4, GT, WSLOT]),
                    op=Alu.is_equal)
                for b in range(4):
                    nc.vector.tensor_tensor(
                        out=madc[:],
                        in0=M[:, b],
                        in1=adrep[:, g * GRP * WSLOT:(g + 1) * GRP * WSLOT]
                            .rearrange("p (w d) -> p w d", w=GRP)
                            [:, :, None, :]
                            .to_broadcast([P, GRP, TPB, WSLOT]),
                        op=Alu.mult)
                    nc.vector.tensor_reduce(
                        out=adpec[:, b * GT:(b + 1) * GT], in_=madc[:],
                        axis=AxL.X, op=Alu.add)
                # per-edge scores [P, 4*GT]
                as_t = sc.tile([P, 4 * GT], F32, tag="as_t")
                nc.vector.tensor_tensor(out=as_t[:],
                                        in0=vslab[:, :, :, 65],
                                        in1=vslab[:, :, :, 66],
                                        op=Alu.add)
                s_t = sc.tile([P, 4 * GT], F32, tag="s_t")
                nc.vector.tensor_tensor(
                    out=s_t[:], in0=as_t[:], in1=adpec[:], op=Alu.add)
                lr = sc.tile([P, 4 * GT], F32, tag="lr")
                nc.vector.scalar_tensor_tensor(
                    out=lr[:], in0=s_t[:], scalar=NEG_SLOPE,
                    in1=s_t[:], op0=Alu.mult, op1=Alu.max)
                ex = sc.tile([P, 4 * GT], F32, tag="ex")
                nc.scalar.activation(ex[:], lr[:], Act.Exp)
                # Mex = M * ex (in place)
                nc.vector.tensor_tensor(
                    out=M[:], in0=M[:],
                    in1=ex[:].rearrange("p (b t) -> p b t", b=4)
                        [:, :, :, None].to_broadcast([P, 4, GT, WSLOT]),
                    op=Alu.mult)
                # aggregation, window pairs interleaved across 2 PSUM banks
                aggsb = ep.tile([WSLOT, GRP, 65], F32, tag="aggsb")
                for wl0 in range(0, GRP, 2):
                    psagg0 = psC.tile([WSLOT, 65], F32)
                    psagg1 = psC.tile([WSLOT, 65], F32)
                    pg = [psagg0, psagg1]
                    tiles = [(b, t) for b in range(4) for t in range(TPB)]
                    for k, (b, t) in enumerate(tiles):
                        for h in range(2):
                            wl = wl0 + h
                            nc.tensor.matmul(
                                pg[h][:],
                                lhsT=M[:, b, wl * TPB + t, :],
                                rhs=vslab[:, b, wl * TPB + t, 0:65],
                                start=(k == 0), stop=(k == 4 * TPB - 1))
                    for h in range(2):
                        nc.scalar.copy(aggsb[:, wl0 + h, :], pg[h][:])
                # batched epilogue over the group's GRP windows
                den = ep.tile([WSLOT, GRP], F32, tag="den")
                nc.vector.tensor_scalar_add(den[:], aggsb[:, :, 64], EPS)
                rec = ep.tile([WSLOT, GRP], F32, tag="rec")
                nc.vector.reciprocal(rec[:], den[:])
                if layer == 0:
                    t1 = ep.tile([WSLOT, GRP, D], BF, tag="t1")
                    nc.vector.tensor_tensor(
                        out=t1[:], in0=aggsb[:, :, 0:64],
                        in1=rec[:, :, None].to_broadcast([WSLOT, GRP, D]),
                        op=Alu.mult)
                    u = ep.tile([WSLOT, GRP, D], BF, tag="u")
                    nc.vector.tensor_tensor(
                        out=u[:], in0=t1[:],
                        in1=b1rep[:, None, :].to_broadcast([WSLOT, GRP, D]),
                        op=Alu.add)
                    for wl0 in range(0, GRP, 2):
                        ps_t = psB.tile([2 * D, WSLOT], F32)
                        nc.tensor.matmul(ps_t[:], lhsT=u[:, wl0:wl0 + 2, :],
                                         rhs=ident[:], start=True, stop=True)
                        for h in range(2):
                            w = g * GRP + wl0 + h
                            nc.scalar.activation(
                                x2T[:, w * WSLOT:(w + 1) * WSLOT],
                                ps_t[h * D:(h + 1) * D, :], Act.Relu)
                else:
                    t1 = ep.tile([WSLOT, GRP, D], F32, tag="t1f")
                    nc.vector.tensor_tensor(
                        out=t1[:], in0=aggsb[:, :, 0:64],
                        in1=rec[:, :, None].to_broadcast([WSLOT, GRP, D]),
                        op=Alu.mult)
                    u = ep.tile([WSLOT, GRP, D], F32, tag="uf")
                    nc.vector.tensor_tensor(
                        out=u[:], in0=t1[:],
                        in1=b2rep[:, None, :].to_broadcast([WSLOT, GRP, D]),
                        op=Alu.add)
                    nc.sync.dma_start(
                        out_d.ap()[g * GRP * WSLOT:(g + 1) * GRP * WSLOT, :]
                            .rearrange("(w s) f -> s w f", w=GRP),
                        u[:])

            # ---- layer 1 ----
            for blk in range(NBLK):
                phase_a_block(0, blk)
            all_gather(0)
            for g in range(NG):
                edge_group(0, g)
                # emit layer-2 phase A for the windows just produced
                for blk in range(g * BPG, (g + 1) * BPG):
                    phase_a_block(1, blk)
            all_gather(1)
            for g in range(NG):
                edge_group(1, g)

    nc.compile()
    return nc


def _host_params(W1, att_src1, att_dst1, b1, W2, att_src2, att_dst2, b2):
    def cat(W, a_s):
        w = np.zeros((D, 67), np.float32)
        w[:, 0:64] = W
        w[:, 65] = W @ np.asarray(a_s, np.float32)
        return w
    w1cat = cat(W1, att_src1)
    w2cat = cat(W2, att_src2).astype(BF16)
    wd1 = np.tile((W1 @ np.asarray(att_dst1, np.float32))[:, None],
                  (1, 128)).astype(np.float32)
    wd2 = np.tile((W2 @ np.asarray(att_dst2, np.float32))[:, None],
                  (1, 128)).astype(BF16)
    b1r = np.tile(np.asarray(b1, np.float32)[None, :], (WSLOT, 1)).astype(BF16)
    b2r = np.tile(np.asarray(b2, np.float32)[None, :], (WSLOT, 1)).astype(np.float32)
    ident = np.eye(WSLOT, dtype=BF16)
    return dict(w1cat=w1cat, w2cat=w2cat, wd1rep=wd1, wd2rep=wd2,
                b1rep=b1r, b2rep=b2r, ident=ident)


def _in_maps(meta, percore, params):
    maps = []
    for c in range(NCORES):
        m = {"xT": percore["xT"][c], "gidx": percore["gidx"][c],
             "dstl": percore["dstl"][c]}
        m.update(params)
        maps.append(m)
    return maps


def kernel(x, edge_index, W1, att_src1, att_dst1, b1, W2, att_src2,
           att_dst2, b2):
    from concourse.bass_utils import run_bass_kernel_spmd

    x = np.asarray(x, np.float32)
    edge_index = np.asarray(edge_index)
    W1 = np.asarray(W1, np.float32)
    W2 = np.asarray(W2, np.float32)

    ek = edge_index.tobytes()
    cached = _CACHE.get("pre")
    if cached is not None and cached[0] == ek and \
            np.array_equal(cached[1], x):
        _, _, meta, percore, node_of_slot = cached
    else:
        meta, percore, node_of_slot = _preprocess(x, edge_index)
        _CACHE["pre"] = (ek, x.copy(), meta, percore, node_of_slot)
    mk = tuple(sorted(meta.items()))
    cached = _CACHE.get("prog")
    if cached is not None and cached[0] == mk:
        nc = cached[1]
    else:
        nc = _build_program(meta)
        _CACHE["prog"] = (mk, nc)
    SLOTS_PC, N = meta["SLOTS_PC"], meta["N"]

    params = _host_params(W1, att_src1, att_dst1, b1, W2, att_src2,
                          att_dst2, b2)
    res = run_bass_kernel_spmd(nc, _in_maps(meta, percore, params),
                               list(range(NCORES)))

    out = np.empty((N, D), np.float32)
    for c in range(NCORES):
        blk = res.results[c]["outS"]  # [SLOTS_PC, 64]
        sl = node_of_slot[c * SLOTS_PC:(c + 1) * SLOTS_PC]
        valid = sl >= 0
        out[sl[valid]] = blk[valid]
    return out


# revision 14
# speedup vs baseline: 1.0922x; 1.0922x over previous
"""Two-layer single-head GAT on Trainium2 (8 NeuronCores, Bass/Tile).

Strategy (graph-parallel over dst nodes), v2:
  - Nodes relabeled into slots: 8 cores x NW windows x 64 slots. Nodes
    assigned to cores balanced by degree (serpentine over degree-sorted
    order), then packed into windows (<=64 nodes, per-src-bucket in-degree
    <= 256; bucket(src) = src_core//2).
  - Per layer each core builds its shard of a node table
    row[n] = [h(64) | 1.0 | as_hi | as_lo] bf16 (128-col rows for the
    256B-stride dma_gather), AllGathered so every core holds the full
    table in DRAM.
  - Edges live on the dst core, grouped by (window-group g of GRP=8
    windows, src bucket b). Per (g,b) one dma_gather of CALL=2048 rows
    (int16 idx within the bucket slice), issued on SWDGE queue b so the
    four buckets' Q7 descriptor generation overlaps across core pairs.
  - Per (g,b), batched DVE ops build M = onehot(dstl) [128e x 16t x 64d],
    Madc = M * adrep (scratch), adpec = rowsum(Madc) = a_d[dst_e]; scores
    s = as_hi+as_lo+adpec, ex = exp(lrelu(s)) (ACT); Mex = M * ex.
  - Aggregation per window: PSUM [64d x 65] += Mex_tile^T @ vslab_tile
    (lhsT = Mex [128e x 64d], rhs = raw gathered rows [128e x 65] =
    [h | 1]); epilogue divides by the ex-sum column (col 64), adds bias;
    layer 1 transposes back to [64f x 64s] via identity matmul + ACT relu
    copy into the resident x2T; layer 2 DMAs slot-major fp32 output.
Softmax max-subtraction is skipped (scores are O(10); exp stays in fp32
range; alpha is mathematically identical). Output unpermuted on host.
"""

import numpy as np
import ml_dtypes

BF16 = ml_dtypes.bfloat16

NCORES = 8
P = 128
D = 64
NEG_SLOPE = 0.2
EPS = 1e-16

WSLOT = 64        # dst slots per window
NODECAP = 64      # max nodes per window
TPB = 2           # 128-edge tiles per (window, bucket)
BCAP = TPB * 128  # per-bucket in-degree cap per window
GRP = 8           # windows per gather group
GT = GRP * TPB    # tiles per (group, bucket)
CALL = GT * 128   # idxs per dma_gather call (keep < ~2500: walrus 16-bit)
FILL = 999.0      # dstl filler (never matches iota 0..63)

_CACHE = {}


def _preprocess(x, edge_index):
    """Host-side partitioning/indexing. Returns per-core input arrays + meta."""
    N = x.shape[0]
    src = edge_index[0].astype(np.int64)
    dst = edge_index[1].astype(np.int64)

    deg = np.bincount(dst, minlength=N)

    # --- assign nodes to cores: serpentine over degree-sorted order ---
    order = np.argsort(-deg, kind="stable")
    core_of_node = np.empty(N, np.int32)
    pat = np.concatenate([np.arange(NCORES), np.arange(NCORES)[::-1]])
    core_of_node[order] = pat[np.arange(N) % (2 * NCORES)]

    bucket_of_node = core_of_node // 2  # 4 buckets of 2 cores each

    deg_b = np.zeros((N, 4), np.int64)
    for b in range(4):
        m = bucket_of_node[src] == b
        deg_b[:, b] = np.bincount(dst[m], minlength=N)

    # --- pack windows per core: <=NODECAP nodes, per-bucket deg <= BCAP ---
    windows = [[] for _ in range(NCORES)]
    for c in range(NCORES):
        nodes_c = order[core_of_node[order] == c]  # degree-sorted
        cur, cur_b = [], np.zeros(4, np.int64)
        for n in nodes_c:
            db = deg_b[n]
            if cur and (len(cur) >= NODECAP or np.any(cur_b + db > BCAP)):
                windows[c].append(cur)
                cur, cur_b = [], np.zeros(4, np.int64)
            cur.append(n)
            cur_b = cur_b + db
        if cur:
            windows[c].append(cur)

    nw_real = max(len(w) for w in windows)
    NG = -(-nw_real // GRP)
    NW = NG * GRP
    SLOTS_PC = NW * WSLOT
    NSLOT = NCORES * SLOTS_PC
    BSZ = NSLOT // 4
    assert BSZ <= 32767, f"int16 gather range exceeded: BSZ={BSZ}"
    NBLK = SLOTS_PC // P  # phase-A blocks of 128 slots

    # --- slot assignment ---
    slot_of_node = np.full(N, -1, np.int64)
    for c in range(NCORES):
        for w, wl in enumerate(windows[c]):
            base = c * SLOTS_PC + w * WSLOT
            slot_of_node[np.asarray(wl, np.int64)] = base + np.arange(len(wl))
    assert (slot_of_node >= 0).all()

    sslot = slot_of_node[src]
    dslot = slot_of_node[dst]
    ecore = (dslot // SLOTS_PC).astype(np.int64)
    ew = (dslot % SLOTS_PC) // WSLOT
    eb = (sslot // BSZ).astype(np.int64)
    edloc = (dslot % WSLOT).astype(np.int64)
    esidx = (sslot % BSZ).astype(np.int64)

    key = ((ecore * NW + ew) * 4 + eb)
    eorder = np.argsort(key, kind="stable")
    key_s = key[eorder]
    cnt = np.bincount(key_s, minlength=NCORES * NW * 4)
    assert cnt.max() <= BCAP, f"bucket cap violated: {cnt.max()}"
    starts = np.zeros(NCORES * NW * 4 + 1, np.int64)
    np.cumsum(cnt, out=starts[1:])
    esidx_s = esidx[eorder]
    edloc_s = edloc[eorder]

    # gidx: [C, NG, 4, CALL]; dloc: [C, NG, 4, GT, 128]
    gidx = np.zeros((NCORES, NG, 4, CALL), np.int16)
    dloc = np.full((NCORES, NG, 4, GT, P), FILL, np.float32)
    for c in range(NCORES):
        for w in range(NW):
            g, wl = divmod(w, GRP)
            for b in range(4):
                k = (c * NW + w) * 4 + b
                lo, hi = starts[k], starts[k + 1]
                n = hi - lo
                if n == 0:
                    continue
                off = wl * TPB * P
                gidx[c, g, b, off:off + n] = esidx_s[lo:hi].astype(np.int16)
                tt = (np.arange(n) // P) + wl * TPB
                pp = np.arange(n) % P
                dloc[c, g, b, tt, pp] = edloc_s[lo:hi].astype(np.float32)

    # wrap-16 + replicate to 128 partitions: [C, NG*4*128, CALL//16]
    g16 = gidx.reshape(NCORES, NG * 4, CALL // 16, 16).transpose(0, 1, 3, 2)
    g128 = np.tile(g16, (1, 1, 8, 1)).reshape(NCORES, NG * 4 * 128, CALL // 16)
    # dstl: [C, 128, NG * 64]  col = g*64 + b*GT + t
    dl = dloc.transpose(0, 4, 1, 2, 3).reshape(NCORES, P, NG * 4 * GT)
    dl = np.ascontiguousarray(dl).astype(BF16)

    node_of_slot = np.full(NSLOT, -1, np.int64)
    node_of_slot[slot_of_node] = np.arange(N)
    xT = np.zeros((NCORES, D, SLOTS_PC), np.float32)
    for c in range(NCORES):
        sl = node_of_slot[c * SLOTS_PC:(c + 1) * SLOTS_PC]
        valid = sl >= 0
        blk = np.zeros((SLOTS_PC, D), np.float32)
        blk[valid] = x[sl[valid]]
        xT[c] = blk.T

    meta = dict(NW=NW, NG=NG, NBLK=NBLK, SLOTS_PC=SLOTS_PC, NSLOT=NSLOT,
                BSZ=BSZ, N=N)
    percore = dict(xT=xT, gidx=g128, dstl=dl)
    return meta, percore, node_of_slot


def _build_program(meta):
    import concourse.bacc as bacc
    import concourse.tile as tile
    from concourse import mybir

    F32, BF, I16 = mybir.dt.float32, mybir.dt.bfloat16, mybir.dt.int16
    Alu = mybir.AluOpType
    Act = mybir.ActivationFunctionType
    AxL = mybir.AxisListType

    NW, NG, NBLK = meta["NW"], meta["NG"], meta["NBLK"]
    SLOTS_PC, NSLOT, BSZ = meta["SLOTS_PC"], meta["NSLOT"], meta["BSZ"]
    BPG = GRP * WSLOT // P  # phase-A blocks per window-group (4)

    nc = bacc.Bacc("TRN2", target_bir_lowering=False, debug=False,
                   num_devices=NCORES, num_swdge_queues=4)

    xT_d = nc.dram_tensor("xT", [D, SLOTS_PC], F32, kind="ExternalInput")
    gidx_d = nc.dram_tensor("gidx", [NG * 4 * 128, CALL // 16], I16,
                            kind="ExternalInput")
    dstl_d = nc.dram_tensor("dstl", [P, NG * 4 * GT], BF,
                            kind="ExternalInput")
    w1cat_d = nc.dram_tensor("w1cat", [D, 67], F32, kind="ExternalInput")
    w2cat_d = nc.dram_tensor("w2cat", [D, 67], BF, kind="ExternalInput")
    wd1_d = nc.dram_tensor("wd1rep", [D, 128], F32, kind="ExternalInput")
    wd2_d = nc.dram_tensor("wd2rep", [D, 128], BF, kind="ExternalInput")
    b1rep_d = nc.dram_tensor("b1rep", [WSLOT, D], BF, kind="ExternalInput")
    b2rep_d = nc.dram_tensor("b2rep", [WSLOT, D], F32, kind="ExternalInput")
    ident_d = nc.dram_tensor("ident", [WSLOT, WSLOT], BF, kind="ExternalInput")
    out_d = nc.dram_tensor("outS", [SLOTS_PC, D], F32, kind="ExternalOutput")

    shard = [nc.dram_tensor(f"shard{l}", [SLOTS_PC, 128], BF) for l in (1, 2)]
    tbl = [nc.dram_tensor(f"tbl{l}", [NSLOT, 128], BF, addr_space="Shared")
           for l in (1, 2)]

    with tile.TileContext(nc) as tc:
        import contextlib
        stack = contextlib.ExitStack()
        with stack:
            const = stack.enter_context(tc.tile_pool(name="const", bufs=1))
            small = stack.enter_context(tc.tile_pool(name="small", bufs=3))
            vp = stack.enter_context(tc.tile_pool(name="vp", bufs=3))
            mp = stack.enter_context(tc.tile_pool(name="mp", bufs=3))
            sc = stack.enter_context(tc.tile_pool(name="sc", bufs=2))
            ep = stack.enter_context(tc.tile_pool(name="ep", bufs=2))
            psA = stack.enter_context(tc.tile_pool(name="psA", bufs=2, space="PSUM"))
            psB = stack.enter_context(tc.tile_pool(name="psB", bufs=2, space="PSUM"))
            psC = stack.enter_context(tc.tile_pool(name="psC", bufs=2, space="PSUM"))

            # constants
            iota_i = const.tile([P, WSLOT], I16)
            nc.gpsimd.iota(iota_i[:], pattern=[[1, WSLOT]], base=0,
                           channel_multiplier=0)
            iota_b = const.tile([P, WSLOT], BF)
            nc.vector.tensor_copy(iota_b[:], iota_i[:])
            w1cat = const.tile([D, 67], F32)
            nc.sync.dma_start(w1cat[:], w1cat_d.ap()[:])
            w2cat = const.tile([D, 67], BF)
            nc.sync.dma_start(w2cat[:], w2cat_d.ap()[:])
            wd1 = const.tile([D, 128], F32)
            nc.sync.dma_start(wd1[:], wd1_d.ap()[:])
            wd2 = const.tile([D, 128], BF)
            nc.sync.dma_start(wd2[:], wd2_d.ap()[:])
            b1rep = const.tile([WSLOT, D], BF)
            nc.sync.dma_start(b1rep[:], b1rep_d.ap()[:])
            b2rep = const.tile([WSLOT, D], F32)
            nc.sync.dma_start(b2rep[:], b2rep_d.ap()[:])
            ident = const.tile([WSLOT, WSLOT], BF)
            nc.sync.dma_start(ident[:], ident_d.ap()[:])

            # resident across layers
            x2T = const.tile([D, SLOTS_PC], BF)
            adrep = const.tile([P, SLOTS_PC], BF)

            def phase_a_block(layer, blk):
                wcat = w1cat if layer == 0 else w2cat
                wdrep = wd1 if layer == 0 else wd2
                shard_l = shard[layer].ap()
                if layer == 0:
                    xtw = small.tile([D, P], F32, tag="xtw")
                    nc.sync.dma_start(xtw[:], xT_d.ap()[:, blk * P:(blk + 1) * P])
                    lhs = xtw[:]
                else:
                    lhs = x2T[:, blk * P:(blk + 1) * P]
                ps_c = psA.tile([P, 67], F32)
                nc.tensor.matmul(ps_c[:], lhsT=lhs, rhs=wcat[:],
                                 start=True, stop=True)
                ps_ad = psB.tile([P, 128], F32)
                nc.tensor.matmul(ps_ad[:], lhsT=wdrep[:], rhs=lhs,
                                 start=True, stop=True)
                tblrow = small.tile([P, 67], BF, tag="tblrow")
                nc.scalar.copy(tblrow[:], ps_c[:])
                nc.vector.memset(tblrow[:, 64:65], 1.0)
                # a_s_lo residual for extra precision
                nc.vector.tensor_tensor(out=tblrow[:, 66:67],
                                        in0=ps_c[:, 65:66],
                                        in1=tblrow[:, 65:66],
                                        op=Alu.subtract)
                nc.sync.dma_start(shard_l[blk * P:(blk + 1) * P, 0:67],
                                  tblrow[:])
                nc.vector.tensor_copy(adrep[:, blk * P:(blk + 1) * P],
                                      ps_ad[:])

            def all_gather(layer):
                nc.gpsimd.collective_compute(
                    "AllGather", mybir.AluOpType.bypass,
                    replica_groups=[list(range(NCORES))],
                    ins=[shard[layer].ap()[:, :]], outs=[tbl[layer].ap()[:, :]],
                )

            def edge_group(layer, g):
                tbl_l = tbl[layer].ap()
                dstl = sc.tile([P, 4 * GT], BF, tag="dstl")
                nc.sync.dma_start(
                    dstl[:],
                    dstl_d.ap()[:, g * 4 * GT:(g + 1) * 4 * GT])
                vslab = vp.tile([P, 4, GT, 128], BF, tag="vslab")
                for b in range(4):
                    idxt = small.tile([128, CALL // 16], I16, tag="idxt")
                    r0 = (g * 4 + b) * 128
                    nc.sync.dma_start(idxt[:], gidx_d.ap()[r0:r0 + 128, :])
                    nc.gpsimd.dma_gather(
                        out_ap=vslab[:, b, :, :],
                        in_ap=tbl_l[b * BSZ:(b + 1) * BSZ, :],
                        idxs_ap=idxt[:], num_idxs=CALL, num_idxs_reg=CALL,
                        elem_size=128, single_packet=False, queue_num=b)

                M = mp.tile([P, 4, GT, WSLOT], BF, tag="M")
                madc = sc.tile([P, GT, WSLOT], BF, tag="madc")
                adpec = sc.tile([P, 4 * GT], F32, tag="adpec")
                nc.vector.tensor_tensor(
                    out=M[:],
                    in0=iota_b[:, None, None, :]
                        .to_broadcast([P, 4, GT, WSLOT]),
                    in1=dstl[:].rearrange("p (b t) -> p b t", b=4)
                        [:, :, :, None].to_broadcast([P, 4, GT, WSLOT]),
                    op=Alu.is_equal)
                for b in range(4):
                    nc.vector.tensor_tensor(
                        out=madc[:],
                        in0=M[:, b],
                        in1=adrep[:, g * GRP * WSLOT:(g + 1) * GRP * WSLOT]
                            .rearrange("p (w d) -> p w d", w=GRP)
                            [:, :, None, :]
                            .to_broadcast([P, GRP, TPB, WSLOT]),
                        op=Alu.mult)
                    nc.vector.tensor_reduce(
                        out=adpec[:, b * GT:(b + 1) * GT], in_=madc[:],
                        axis=AxL.X, op=Alu.add)
                # per-edge scores [P, 4*GT]
                as_t = sc.tile([P, 4 * GT], F32, tag="as_t")
                nc.vector.tensor_tensor(out=as_t[:],
                                        in0=vslab[:, :, :, 65],
                                        in1=vslab[:, :, :, 66],
                                        op=Alu.add)
                s_t = sc.tile([P, 4 * GT], F32, tag="s_t")
                nc.vector.tensor_tensor(
                    out=s_t[:], in0=as_t[:], in1=adpec[:], op=Alu.add)
                lr = sc.tile([P, 4 * GT], F32, tag="lr")
                nc.vector.scalar_tensor_tensor(
                    out=lr[:], in0=s_t[:], scalar=NEG_SLOPE,
                    in1=s_t[:], op0=Alu.mult, op1=Alu.max)
                ex = sc.tile([P, 4 * GT], F32, tag="ex")
                nc.scalar.activation(ex[:], lr[:], Act.Exp)
                # Mex = M * ex (in place)
                nc.vector.tensor_tensor(
                    out=M[:], in0=M[:],
                    in1=ex[:].rearrange("p (b t) -> p b t", b=4)
                        [:, :, :, None].to_broadcast([P, 4, GT, WSLOT]),
                    op=Alu.mult)
                # aggregation per window into PSUM, collected into aggsb
                aggsb = ep.tile([WSLOT, GRP, 65], F32, tag="aggsb")
                for wl in range(GRP):
                    psagg = psC.tile([WSLOT, 65], F32)
                    k = 0
                    for b in range(4):
                        for t in range(TPB):
                            nc.tensor.matmul(
                                psagg[:],
                                lhsT=M[:, b, wl * TPB + t, :],
                                rhs=vslab[:, b, wl * TPB + t, 0:65],
                                start=(k == 0), stop=(k == 4 * TPB - 1))
                            k += 1
                    nc.scalar.copy(aggsb[:, wl, :], psagg[:])
                # batched epilogue over the group's GRP windows
                den = ep.tile([WSLOT, GRP], F32, tag="den")
                nc.vector.tensor_scalar_add(den[:], aggsb[:, :, 64], EPS)
                rec = ep.tile([WSLOT, GRP], F32, tag="rec")
                nc.vector.reciprocal(rec[:], den[:])
                if layer == 0:
                    t1 = ep.tile([WSLOT, GRP, D], BF, tag="t1")
                    nc.vector.tensor_tensor(
                        out=t1[:], in0=aggsb[:, :, 0:64],
                        in1=rec[:, :, None].to_broadcast([WSLOT, GRP, D]),
                        op=Alu.mult)
                    u = ep.tile([WSLOT, GRP, D], BF, tag="u")
                    nc.vector.tensor_tensor(
                        out=u[:], in0=t1[:],
                        in1=b1rep[:, None, :].to_broadcast([WSLOT, GRP, D]),
                        op=Alu.add)
                    for wl0 in range(0, GRP, 2):
                        ps_t = psB.tile([2 * D, WSLOT], F32)
                        nc.tensor.matmul(ps_t[:], lhsT=u[:, wl0:wl0 + 2, :],
                                         rhs=ident[:], start=True, stop=True)
                        for h in range(2):
                            w = g * GRP + wl0 + h
                            nc.scalar.activation(
                                x2T[:, w * WSLOT:(w + 1) * WSLOT],
                                ps_t[h * D:(h + 1) * D, :], Act.Relu)
                else:
                    t1 = ep.tile([WSLOT, GRP, D], F32, tag="t1f")
                    nc.vector.tensor_tensor(
                        out=t1[:], in0=aggsb[:, :, 0:64],
                        in1=rec[:, :, None].to_broadcast([WSLOT, GRP, D]),
                        op=Alu.mult)
                    u = ep.tile([WSLOT, GRP, D], F32, tag="uf")
                    nc.vector.tensor_tensor(
                        out=u[:], in0=t1[:],
                        in1=b2rep[:, None, :].to_broadcast([WSLOT, GRP, D]),
                        op=Alu.add)
                    nc.sync.dma_start(
                        out_d.ap()[g * GRP * WSLOT:(g + 1) * GRP * WSLOT, :]
                            .rearrange("(w s) f -> s w f", w=GRP),
                        u[:])

            # ---- layer 1 ----
            for blk in range(NBLK):
                phase_a_block(0, blk)
            all_gather(0)
            for g in range(NG):
                edge_group(0, g)
                # emit layer-2 phase A for the windows just produced
                for blk in range(g * BPG, (g + 1) * BPG):
                    phase_a_block(1, blk)
            all_gather(1)
            for g in range(NG):
                edge_group(1, g)

    nc.compile()
    return nc


def _host_params(W1, att_src1, att_dst1, b1, W2, att_src2, att_dst2, b2):
    def cat(W, a_s):
        w = np.zeros((D, 67), np.float32)
        w[:, 0:64] = W
        w[:, 65] = W @ np.asarray(a_s, np.float32)
        return w
    w1cat = cat(W1, att_src1)
    w2cat = cat(W2, att_src2).astype(BF16)
    wd1 = np.tile((W1 @ np.asarray(att_dst1, np.float32))[:, None],
                  (1, 128)).astype(np.float32)
    wd2 = np.tile((W2 @ np.asarray(att_dst2, np.float32))[:, None],
                  (1, 128)).astype(BF16)
    b1r = np.tile(np.asarray(b1, np.float32)[None, :], (WSLOT, 1)).astype(BF16)
    b2r = np.tile(np.asarray(b2, np.float32)[None, :], (WSLOT, 1)).astype(np.float32)
    ident = np.eye(WSLOT, dtype=BF16)
    return dict(w1cat=w1cat, w2cat=w2cat, wd1rep=wd1, wd2rep=wd2,
                b1rep=b1r, b2rep=b2r, ident=ident)


def _in_maps(meta, percore, params):
    maps = []
    for c in range(NCORES):
        m = {"xT": percore["xT"][c], "gidx": percore["gidx"][c],
             "dstl": percore["dstl"][c]}
        m.update(params)
        maps.append(m)
    return maps


def kernel(x, edge_index, W1, att_src1, att_dst1, b1, W2, att_src2,
           att_dst2, b2):
    from concourse.bass_utils import run_bass_kernel_spmd

    x = np.asarray(x, np.float32)
    edge_index = np.asarray(edge_index)
    W1 = np.asarray(W1, np.float32)
    W2 = np.asarray(W2, np.float32)

    ek = edge_index.tobytes()
    cached = _CACHE.get("pre")
    if cached is not None and cached[0] == ek and \
            np.array_equal(cached[1], x):
        _, _, meta, percore, node_of_slot = cached
    else:
        meta, percore, node_of_slot = _preprocess(x, edge_index)
        _CACHE["pre"] = (ek, x.copy(), meta, percore, node_of_slot)
    mk = tuple(sorted(meta.items()))
    cached = _CACHE.get("prog")
    if cached is not None and cached[0] == mk:
        nc = cached[1]
    else:
        nc = _build_program(meta)
        _CACHE["prog"] = (mk, nc)
    SLOTS_PC, N = meta["SLOTS_PC"], meta["N"]

    params = _host_params(W1, att_src1, att_dst1, b1, W2, att_src2,
                          att_dst2, b2)
    res = run_bass_kernel_spmd(nc, _in_maps(meta, percore, params),
                               list(range(NCORES)))

    out = np.empty((N, D), np.float32)
    for c in range(NCORES):
        blk = res.results[c]["outS"]  # [SLOTS_PC, 64]
        sl = node_of_slot[c * SLOTS_PC:(c + 1) * SLOTS_PC]
        valid = sl >= 0
        out[sl[valid]] = blk[valid]
    return out


# revision 15
# speedup vs baseline: 1.2525x; 1.1468x over previous
"""Two-layer single-head GAT on Trainium2 (8 NeuronCores, Bass/Tile).

Strategy (graph-parallel over dst nodes), v2:
  - Nodes relabeled into slots: 8 cores x NW windows x 64 slots. Nodes
    assigned to cores balanced by degree (serpentine over degree-sorted
    order), then packed into windows (<=64 nodes, per-src-bucket in-degree
    <= 256; bucket(src) = src_core//2).
  - Per layer each core builds its shard of a node table
    row[n] = [h(64) | 1.0 | as_hi | as_lo] bf16 (128-col rows for the
    256B-stride dma_gather), AllGathered so every core holds the full
    table in DRAM.
  - Edges live on the dst core, grouped by (window-group g of GRP=8
    windows, src bucket b). Per (g,b) one dma_gather of CALL=2048 rows
    (int16 idx within the bucket slice), issued on SWDGE queue b so the
    four buckets' Q7 descriptor generation overlaps across core pairs.
  - Per (g,b), batched DVE ops build M = onehot(dstl) [128e x 16t x 64d],
    Madc = M * adrep (scratch), adpec = rowsum(Madc) = a_d[dst_e]; scores
    s = as_hi+as_lo+adpec, ex = exp(lrelu(s)) (ACT); Mex = M * ex.
  - Aggregation per window: PSUM [64d x 65] += Mex_tile^T @ vslab_tile
    (lhsT = Mex [128e x 64d], rhs = raw gathered rows [128e x 65] =
    [h | 1]); epilogue divides by the ex-sum column (col 64), adds bias;
    layer 1 transposes back to [64f x 64s] via identity matmul + ACT relu
    copy into the resident x2T; layer 2 DMAs slot-major fp32 output.
Softmax max-subtraction is skipped (scores are O(10); exp stays in fp32
range; alpha is mathematically identical). Output unpermuted on host.
"""

import numpy as np
import ml_dtypes

BF16 = ml_dtypes.bfloat16

NCORES = 8
P = 128
D = 64
NEG_SLOPE = 0.2
EPS = 1e-16

WSLOT = 64        # dst slots per window
NODECAP = 64      # max nodes per window
TPB = 2           # 128-edge tiles per (window, bucket)
BCAP = TPB * 128  # per-bucket in-degree cap per window
GRP = 8           # windows per gather group
GT = GRP * TPB    # tiles per (group, bucket)
CALL = GT * 128   # idxs per dma_gather call (keep < ~2500: walrus 16-bit)
FILL = 999.0      # dstl filler (never matches iota 0..63)

_CACHE = {}


def _preprocess(x, edge_index):
    """Host-side partitioning/indexing. Returns per-core input arrays + meta."""
    N = x.shape[0]
    src = edge_index[0].astype(np.int64)
    dst = edge_index[1].astype(np.int64)

    deg = np.bincount(dst, minlength=N)

    # --- assign nodes to cores: serpentine over degree-sorted order ---
    order = np.argsort(-deg, kind="stable")
    core_of_node = np.empty(N, np.int32)
    pat = np.concatenate([np.arange(NCORES), np.arange(NCORES)[::-1]])
    core_of_node[order] = pat[np.arange(N) % (2 * NCORES)]

    bucket_of_node = core_of_node // 2  # 4 buckets of 2 cores each

    deg_b = np.zeros((N, 4), np.int64)
    for b in range(4):
        m = bucket_of_node[src] == b
        deg_b[:, b] = np.bincount(dst[m], minlength=N)

    # --- pack windows per core: <=NODECAP nodes, per-bucket deg <= BCAP ---
    windows = [[] for _ in range(NCORES)]
    for c in range(NCORES):
        nodes_c = order[core_of_node[order] == c]  # degree-sorted
        cur, cur_b = [], np.zeros(4, np.int64)
        for n in nodes_c:
            db = deg_b[n]
            if cur and (len(cur) >= NODECAP or np.any(cur_b + db > BCAP)):
                windows[c].append(cur)
                cur, cur_b = [], np.zeros(4, np.int64)
            cur.append(n)
            cur_b = cur_b + db
        if cur:
            windows[c].append(cur)

    nw_real = max(len(w) for w in windows)
    NG = -(-nw_real // GRP)
    NW = NG * GRP
    SLOTS_PC = NW * WSLOT
    NSLOT = NCORES * SLOTS_PC
    BSZ = NSLOT // 4
    assert BSZ <= 32767, f"int16 gather range exceeded: BSZ={BSZ}"
    NBLK = SLOTS_PC // P  # phase-A blocks of 128 slots

    # --- slot assignment ---
    slot_of_node = np.full(N, -1, np.int64)
    for c in range(NCORES):
        for w, wl in enumerate(windows[c]):
            base = c * SLOTS_PC + w * WSLOT
            slot_of_node[np.asarray(wl, np.int64)] = base + np.arange(len(wl))
    assert (slot_of_node >= 0).all()

    sslot = slot_of_node[src]
    dslot = slot_of_node[dst]
    ecore = (dslot // SLOTS_PC).astype(np.int64)
    ew = (dslot % SLOTS_PC) // WSLOT
    eb = (sslot // BSZ).astype(np.int64)
    edloc = (dslot % WSLOT).astype(np.int64)
    esidx = (sslot % BSZ).astype(np.int64)

    key = ((ecore * NW + ew) * 4 + eb)
    eorder = np.argsort(key, kind="stable")
    key_s = key[eorder]
    cnt = np.bincount(key_s, minlength=NCORES * NW * 4)
    assert cnt.max() <= BCAP, f"bucket cap violated: {cnt.max()}"
    starts = np.zeros(NCORES * NW * 4 + 1, np.int64)
    np.cumsum(cnt, out=starts[1:])
    esidx_s = esidx[eorder]
    edloc_s = edloc[eorder]

    # gidx: [C, NG, 4, CALL]; dloc: [C, NG, 4, GT, 128]
    gidx = np.zeros((NCORES, NG, 4, CALL), np.int16)
    dloc = np.full((NCORES, NG, 4, GT, P), FILL, np.float32)
    for c in range(NCORES):
        for w in range(NW):
            g, wl = divmod(w, GRP)
            for b in range(4):
                k = (c * NW + w) * 4 + b
                lo, hi = starts[k], starts[k + 1]
                n = hi - lo
                if n == 0:
                    continue
                off = wl * TPB * P
                gidx[c, g, b, off:off + n] = esidx_s[lo:hi].astype(np.int16)
                tt = (np.arange(n) // P) + wl * TPB
                pp = np.arange(n) % P
                dloc[c, g, b, tt, pp] = edloc_s[lo:hi].astype(np.float32)

    # wrap-16 + replicate to 128 partitions: [C, NG*4*128, CALL//16]
    g16 = gidx.reshape(NCORES, NG * 4, CALL // 16, 16).transpose(0, 1, 3, 2)
    g128 = np.tile(g16, (1, 1, 8, 1)).reshape(NCORES, NG * 4 * 128, CALL // 16)
    # dstl: [C, 128, NG * 64]  col = g*64 + b*GT + t
    dl = dloc.transpose(0, 4, 1, 2, 3).reshape(NCORES, P, NG * 4 * GT)
    dl = np.ascontiguousarray(dl).astype(BF16)

    node_of_slot = np.full(NSLOT, -1, np.int64)
    node_of_slot[slot_of_node] = np.arange(N)
    xT = np.zeros((NCORES, D, SLOTS_PC), np.float32)
    for c in range(NCORES):
        sl = node_of_slot[c * SLOTS_PC:(c + 1) * SLOTS_PC]
        valid = sl >= 0
        blk = np.zeros((SLOTS_PC, D), np.float32)
        blk[valid] = x[sl[valid]]
        xT[c] = blk.T

    meta = dict(NW=NW, NG=NG, NBLK=NBLK, SLOTS_PC=SLOTS_PC, NSLOT=NSLOT,
                BSZ=BSZ, N=N)
    percore = dict(xT=xT, gidx=g128, dstl=dl)
    return meta, percore, node_of_slot


def _build_program(meta):
    import concourse.bacc as bacc
    import concourse.tile as tile
    from concourse import mybir

    F32, BF, I16 = mybir.dt.float32, mybir.dt.bfloat16, mybir.dt.int16
    Alu = mybir.AluOpType
    Act = mybir.ActivationFunctionType
    AxL = mybir.AxisListType

    NW, NG, NBLK = meta["NW"], meta["NG"], meta["NBLK"]
    SLOTS_PC, NSLOT, BSZ = meta["SLOTS_PC"], meta["NSLOT"], meta["BSZ"]
    BPG = GRP * WSLOT // P  # phase-A blocks per window-group (4)

    nc = bacc.Bacc("TRN2", target_bir_lowering=False, debug=False,
                   num_devices=NCORES, num_swdge_queues=4)

    xT_d = nc.dram_tensor("xT", [D, SLOTS_PC], F32, kind="ExternalInput")
    gidx_d = nc.dram_tensor("gidx", [NG * 4 * 128, CALL // 16], I16,
                            kind="ExternalInput")
    dstl_d = nc.dram_tensor("dstl", [P, NG * 4 * GT], BF,
                            kind="ExternalInput")
    w1cat_d = nc.dram_tensor("w1cat", [D, 67], F32, kind="ExternalInput")
    w2cat_d = nc.dram_tensor("w2cat", [D, 67], BF, kind="ExternalInput")
    wd1_d = nc.dram_tensor("wd1rep", [D, 128], F32, kind="ExternalInput")
    wd2_d = nc.dram_tensor("wd2rep", [D, 128], BF, kind="ExternalInput")
    b1rep_d = nc.dram_tensor("b1rep", [WSLOT, D], BF, kind="ExternalInput")
    b2rep_d = nc.dram_tensor("b2rep", [WSLOT, D], F32, kind="ExternalInput")
    ident_d = nc.dram_tensor("ident", [WSLOT, WSLOT], BF, kind="ExternalInput")
    out_d = nc.dram_tensor("outS", [SLOTS_PC, D], F32, kind="ExternalOutput")

    shard = [nc.dram_tensor(f"shard{l}", [SLOTS_PC, 128], BF) for l in (1, 2)]
    tbl = [nc.dram_tensor(f"tbl{l}", [NSLOT, 128], BF, addr_space="Shared")
           for l in (1, 2)]

    with tile.TileContext(nc) as tc:
        import contextlib
        stack = contextlib.ExitStack()
        with stack:
            const = stack.enter_context(tc.tile_pool(name="const", bufs=1))
            small = stack.enter_context(tc.tile_pool(name="small", bufs=3))
            vp = stack.enter_context(tc.tile_pool(name="vp", bufs=3))
            mp = stack.enter_context(tc.tile_pool(name="mp", bufs=3))
            sc = stack.enter_context(tc.tile_pool(name="sc", bufs=2))
            ep = stack.enter_context(tc.tile_pool(name="ep", bufs=2))
            psA = stack.enter_context(tc.tile_pool(name="psA", bufs=2, space="PSUM"))
            psB = stack.enter_context(tc.tile_pool(name="psB", bufs=2, space="PSUM"))
            psC = stack.enter_context(tc.tile_pool(name="psC", bufs=2, space="PSUM"))

            # constants
            iota_i = const.tile([P, WSLOT], I16)
            nc.gpsimd.iota(iota_i[:], pattern=[[1, WSLOT]], base=0,
                           channel_multiplier=0)
            iota_b = const.tile([P, WSLOT], BF)
            nc.vector.tensor_copy(iota_b[:], iota_i[:])
            w1cat = const.tile([D, 67], F32)
            nc.sync.dma_start(w1cat[:], w1cat_d.ap()[:])
            w2cat = const.tile([D, 67], BF)
            nc.sync.dma_start(w2cat[:], w2cat_d.ap()[:])
            wd1 = const.tile([D, 128], F32)
            nc.sync.dma_start(wd1[:], wd1_d.ap()[:])
            wd2 = const.tile([D, 128], BF)
            nc.sync.dma_start(wd2[:], wd2_d.ap()[:])
            b1rep = const.tile([WSLOT, D], BF)
            nc.sync.dma_start(b1rep[:], b1rep_d.ap()[:])
            b2rep = const.tile([WSLOT, D], F32)
            nc.sync.dma_start(b2rep[:], b2rep_d.ap()[:])
            ident = const.tile([WSLOT, WSLOT], BF)
            nc.sync.dma_start(ident[:], ident_d.ap()[:])

            # resident across layers
            x2T = const.tile([D, SLOTS_PC], BF)
            adrep = const.tile([P, SLOTS_PC], BF)

            def phase_a_block(layer, blk):
                wcat = w1cat if layer == 0 else w2cat
                wdrep = wd1 if layer == 0 else wd2
                shard_l = shard[layer].ap()
                if layer == 0:
                    xtw = small.tile([D, P], F32, tag="xtw")
                    nc.sync.dma_start(xtw[:], xT_d.ap()[:, blk * P:(blk + 1) * P])
                    lhs = xtw[:]
                else:
                    lhs = x2T[:, blk * P:(blk + 1) * P]
                ps_c = psA.tile([P, 67], F32)
                nc.tensor.matmul(ps_c[:], lhsT=lhs, rhs=wcat[:],
                                 start=True, stop=True)
                ps_ad = psB.tile([P, 128], F32)
                nc.tensor.matmul(ps_ad[:], lhsT=wdrep[:], rhs=lhs,
                                 start=True, stop=True)
                tblrow = small.tile([P, 67], BF, tag="tblrow")
                nc.scalar.copy(tblrow[:], ps_c[:])
                nc.vector.memset(tblrow[:, 64:65], 1.0)
                # a_s_lo residual for extra precision
                nc.vector.tensor_tensor(out=tblrow[:, 66:67],
                                        in0=ps_c[:, 65:66],
                                        in1=tblrow[:, 65:66],
                                        op=Alu.subtract)
                nc.scalar.dma_start(shard_l[blk * P:(blk + 1) * P, 0:67],
                                    tblrow[:])
                nc.scalar.copy(adrep[:, blk * P:(blk + 1) * P], ps_ad[:])

            def all_gather(layer):
                nc.gpsimd.collective_compute(
                    "AllGather", mybir.AluOpType.bypass,
                    replica_groups=[list(range(NCORES))],
                    ins=[shard[layer].ap()[:, :]], outs=[tbl[layer].ap()[:, :]],
                )

            def edge_group(layer, g):
                tbl_l = tbl[layer].ap()
                dstl = sc.tile([P, 4 * GT], BF, tag="dstl")
                nc.sync.dma_start(
                    dstl[:],
                    dstl_d.ap()[:, g * 4 * GT:(g + 1) * 4 * GT])
                vslab = vp.tile([P, 4, GT, 128], BF, tag="vslab")
                for b in range(4):
                    idxt = small.tile([128, CALL // 16], I16, tag="idxt")
                    r0 = (g * 4 + b) * 128
                    nc.sync.dma_start(idxt[:], gidx_d.ap()[r0:r0 + 128, :])
                    nc.gpsimd.dma_gather(
                        out_ap=vslab[:, b, :, :],
                        in_ap=tbl_l[b * BSZ:(b + 1) * BSZ, :],
                        idxs_ap=idxt[:], num_idxs=CALL, num_idxs_reg=CALL,
                        elem_size=128, single_packet=False, queue_num=b)

                M = mp.tile([P, 4, GT, WSLOT], BF, tag="M")
                madc = sc.tile([P, GT, WSLOT], BF, tag="madc")
                adpec = sc.tile([P, 4 * GT], F32, tag="adpec")
                nc.vector.tensor_tensor(
                    out=M[:],
                    in0=iota_b[:, None, None, :]
                        .to_broadcast([P, 4, GT, WSLOT]),
                    in1=dstl[:].rearrange("p (b t) -> p b t", b=4)
                        [:, :, :, None].to_broadcast([P, 4, GT, WSLOT]),
                    op=Alu.is_equal)
                for b in range(4):
                    nc.vector.tensor_tensor(
                        out=madc[:],
                        in0=M[:, b],
                        in1=adrep[:, g * GRP * WSLOT:(g + 1) * GRP * WSLOT]
                            .rearrange("p (w d) -> p w d", w=GRP)
                            [:, :, None, :]
                            .to_broadcast([P, GRP, TPB, WSLOT]),
                        op=Alu.mult)
                    nc.vector.tensor_reduce(
                        out=adpec[:, b * GT:(b + 1) * GT], in_=madc[:],
                        axis=AxL.X, op=Alu.add)
                # per-edge scores [P, 4*GT]
                as_t = sc.tile([P, 4 * GT], F32, tag="as_t")
                nc.vector.tensor_tensor(out=as_t[:],
                                        in0=vslab[:, :, :, 65],
                                        in1=vslab[:, :, :, 66],
                                        op=Alu.add)
                s_t = sc.tile([P, 4 * GT], F32, tag="s_t")
                nc.vector.tensor_tensor(
                    out=s_t[:], in0=as_t[:], in1=adpec[:], op=Alu.add)
                lr = sc.tile([P, 4 * GT], F32, tag="lr")
                nc.vector.scalar_tensor_tensor(
                    out=lr[:], in0=s_t[:], scalar=NEG_SLOPE,
                    in1=s_t[:], op0=Alu.mult, op1=Alu.max)
                ex = sc.tile([P, 4 * GT], F32, tag="ex")
                nc.scalar.activation(ex[:], lr[:], Act.Exp)
                # Mex = M * ex (in place)
                nc.vector.tensor_tensor(
                    out=M[:], in0=M[:],
                    in1=ex[:].rearrange("p (b t) -> p b t", b=4)
                        [:, :, :, None].to_broadcast([P, 4, GT, WSLOT]),
                    op=Alu.mult)
                # aggregation per window into PSUM, collected into aggsb
                aggsb = ep.tile([WSLOT, GRP, 65], F32, tag="aggsb")
                for wl in range(GRP):
                    psagg = psC.tile([WSLOT, 65], F32)
                    k = 0
                    for b in range(4):
                        for t in range(TPB):
                            nc.tensor.matmul(
                                psagg[:],
                                lhsT=M[:, b, wl * TPB + t, :],
                                rhs=vslab[:, b, wl * TPB + t, 0:65],
                                start=(k == 0), stop=(k == 4 * TPB - 1))
                            k += 1
                    nc.scalar.copy(aggsb[:, wl, :], psagg[:])
                # batched epilogue over the group's GRP windows
                den = ep.tile([WSLOT, GRP], F32, tag="den")
                nc.vector.tensor_scalar_add(den[:], aggsb[:, :, 64], EPS)
                rec = ep.tile([WSLOT, GRP], F32, tag="rec")
                nc.vector.reciprocal(rec[:], den[:])
                if layer == 0:
                    t1 = ep.tile([WSLOT, GRP, D], BF, tag="t1")
                    nc.vector.tensor_tensor(
                        out=t1[:], in0=aggsb[:, :, 0:64],
                        in1=rec[:, :, None].to_broadcast([WSLOT, GRP, D]),
                        op=Alu.mult)
                    u = ep.tile([WSLOT, GRP, D], BF, tag="u")
                    nc.vector.tensor_tensor(
                        out=u[:], in0=t1[:],
                        in1=b1rep[:, None, :].to_broadcast([WSLOT, GRP, D]),
                        op=Alu.add)
                    for wl in range(GRP):
                        w = g * GRP + wl
                        ps_t = psB.tile([D, WSLOT], F32)
                        nc.tensor.matmul(ps_t[:], lhsT=u[:, wl, :],
                                         rhs=ident[:], start=True, stop=True)
                        nc.scalar.activation(
                            x2T[:, w * WSLOT:(w + 1) * WSLOT], ps_t[:],
                            Act.Relu)
                else:
                    t1 = ep.tile([WSLOT, GRP, D], F32, tag="t1f")
                    nc.vector.tensor_tensor(
                        out=t1[:], in0=aggsb[:, :, 0:64],
                        in1=rec[:, :, None].to_broadcast([WSLOT, GRP, D]),
                        op=Alu.mult)
                    u = ep.tile([WSLOT, GRP, D], F32, tag="uf")
                    nc.vector.tensor_tensor(
                        out=u[:], in0=t1[:],
                        in1=b2rep[:, None, :].to_broadcast([WSLOT, GRP, D]),
                        op=Alu.add)
                    nc.sync.dma_start(
                        out_d.ap()[g * GRP * WSLOT:(g + 1) * GRP * WSLOT, :]
                            .rearrange("(w s) f -> s w f", w=GRP),
                        u[:])

            # ---- layer 1 ----
            for blk in range(NBLK):
                phase_a_block(0, blk)
            all_gather(0)
            for g in range(NG):
                edge_group(0, g)
                # emit layer-2 phase A for the windows just produced
                for blk in range(g * BPG, (g + 1) * BPG):
                    phase_a_block(1, blk)
            all_gather(1)
            for g in range(NG):
                edge_group(1, g)

    nc.compile()
    return nc


def _host_params(W1, att_src1, att_dst1, b1, W2, att_src2, att_dst2, b2):
    def cat(W, a_s):
        w = np.zeros((D, 67), np.float32)
        w[:, 0:64] = W
        w[:, 65] = W @ np.asarray(a_s, np.float32)
        return w
    w1cat = cat(W1, att_src1)
    w2cat = cat(W2, att_src2).astype(BF16)
    wd1 = np.tile((W1 @ np.asarray(att_dst1, np.float32))[:, None],
                  (1, 128)).astype(np.float32)
    wd2 = np.tile((W2 @ np.asarray(att_dst2, np.float32))[:, None],
                  (1, 128)).astype(BF16)
    b1r = np.tile(np.asarray(b1, np.float32)[None, :], (WSLOT, 1)).astype(BF16)
    b2r = np.tile(np.asarray(b2, np.float32)[None, :], (WSLOT, 1)).astype(np.float32)
    ident = np.eye(WSLOT, dtype=BF16)
    return dict(w1cat=w1cat, w2cat=w2cat, wd1rep=wd1, wd2rep=wd2,
                b1rep=b1r, b2rep=b2r, ident=ident)


def _in_maps(meta, percore, params):
    maps = []
    for c in range(NCORES):
        m = {"xT": percore["xT"][c], "gidx": percore["gidx"][c],
             "dstl": percore["dstl"][c]}
        m.update(params)
        maps.append(m)
    return maps


def kernel(x, edge_index, W1, att_src1, att_dst1, b1, W2, att_src2,
           att_dst2, b2):
    from concourse.bass_utils import run_bass_kernel_spmd

    x = np.asarray(x, np.float32)
    edge_index = np.asarray(edge_index)
    W1 = np.asarray(W1, np.float32)
    W2 = np.asarray(W2, np.float32)

    ek = edge_index.tobytes()
    cached = _CACHE.get("pre")
    if cached is not None and cached[0] == ek and \
            np.array_equal(cached[1], x):
        _, _, meta, percore, node_of_slot = cached
    else:
        meta, percore, node_of_slot = _preprocess(x, edge_index)
        _CACHE["pre"] = (ek, x.copy(), meta, percore, node_of_slot)
    mk = tuple(sorted(meta.items()))
    cached = _CACHE.get("prog")
    if cached is not None and cached[0] == mk:
        nc = cached[1]
    else:
        nc = _build_program(meta)
        _CACHE["prog"] = (mk, nc)
    SLOTS_PC, N = meta["SLOTS_PC"], meta["N"]

    params = _host_params(W1, att_src1, att_dst1, b1, W2, att_src2,
                          att_dst2, b2)
    res = run_bass_kernel_spmd(nc, _in_maps(meta, percore, params),
                               list(range(NCORES)))

    out = np.empty((N, D), np.float32)
    for c in range(NCORES):
        blk = res.results[c]["outS"]  # [SLOTS_PC, 64]
        sl = node_of_slot[c * SLOTS_PC:(c + 1) * SLOTS_PC]
        valid = sl >= 0
        out[sl[valid]] = blk[valid]
    return out


# revision 16
# speedup vs baseline: 1.2672x; 1.0117x over previous
"""Two-layer single-head GAT on Trainium2 (8 NeuronCores, Bass/Tile).

Strategy (graph-parallel over dst nodes), v2:
  - Nodes relabeled into slots: 8 cores x NW windows x 64 slots. Nodes
    assigned to cores balanced by degree (serpentine over degree-sorted
    order), then packed into windows (<=64 nodes, per-src-bucket in-degree
    <= 256; bucket(src) = src_core//2).
  - Per layer each core builds its shard of a node table
    row[n] = [h(64) | 1.0 | as_hi | as_lo] bf16 (128-col rows for the
    256B-stride dma_gather), AllGathered so every core holds the full
    table in DRAM.
  - Edges live on the dst core, grouped by (window-group g of GRP=8
    windows, src bucket b). Per (g,b) one dma_gather of CALL=2048 rows
    (int16 idx within the bucket slice), issued on SWDGE queue b so the
    four buckets' Q7 descriptor generation overlaps across core pairs.
  - Per (g,b), batched DVE ops build M = onehot(dstl) [128e x 16t x 64d],
    Madc = M * adrep (scratch), adpec = rowsum(Madc) = a_d[dst_e]; scores
    s = as_hi+as_lo+adpec, ex = exp(lrelu(s)) (ACT); Mex = M * ex.
  - Aggregation per window: PSUM [64d x 65] += Mex_tile^T @ vslab_tile
    (lhsT = Mex [128e x 64d], rhs = raw gathered rows [128e x 65] =
    [h | 1]); epilogue divides by the ex-sum column (col 64), adds bias;
    layer 1 transposes back to [64f x 64s] via identity matmul + ACT relu
    copy into the resident x2T; layer 2 DMAs slot-major fp32 output.
Softmax max-subtraction is skipped (scores are O(10); exp stays in fp32
range; alpha is mathematically identical). Output unpermuted on host.
"""

import numpy as np
import ml_dtypes

BF16 = ml_dtypes.bfloat16

NCORES = 8
P = 128
D = 64
NEG_SLOPE = 0.2
EPS = 1e-16

WSLOT = 64        # dst slots per window
NODECAP = 64      # max nodes per window
TPB = 2           # 128-edge tiles per (window, bucket)
BCAP = TPB * 128  # per-bucket in-degree cap per window
GRP = 8           # windows per gather group
GT = GRP * TPB    # tiles per (group, bucket)
CALL = GT * 128   # idxs per dma_gather call (keep < ~2500: walrus 16-bit)
FILL = 999.0      # dstl filler (never matches iota 0..63)

_CACHE = {}


def _preprocess(x, edge_index):
    """Host-side partitioning/indexing. Returns per-core input arrays + meta."""
    N = x.shape[0]
    src = edge_index[0].astype(np.int64)
    dst = edge_index[1].astype(np.int64)

    deg = np.bincount(dst, minlength=N)

    # --- assign nodes to cores: serpentine over degree-sorted order ---
    order = np.argsort(-deg, kind="stable")
    core_of_node = np.empty(N, np.int32)
    pat = np.concatenate([np.arange(NCORES), np.arange(NCORES)[::-1]])
    core_of_node[order] = pat[np.arange(N) % (2 * NCORES)]

    bucket_of_node = core_of_node // 2  # 4 buckets of 2 cores each

    deg_b = np.zeros((N, 4), np.int64)
    for b in range(4):
        m = bucket_of_node[src] == b
        deg_b[:, b] = np.bincount(dst[m], minlength=N)

    # --- pack windows per core: <=NODECAP nodes, per-bucket deg <= BCAP ---
    windows = [[] for _ in range(NCORES)]
    for c in range(NCORES):
        nodes_c = order[core_of_node[order] == c]  # degree-sorted
        cur, cur_b = [], np.zeros(4, np.int64)
        for n in nodes_c:
            db = deg_b[n]
            if cur and (len(cur) >= NODECAP or np.any(cur_b + db > BCAP)):
                windows[c].append(cur)
                cur, cur_b = [], np.zeros(4, np.int64)
            cur.append(n)
            cur_b = cur_b + db
        if cur:
            windows[c].append(cur)

    nw_real = max(len(w) for w in windows)
    NG = -(-nw_real // GRP)
    NW = NG * GRP
    SLOTS_PC = NW * WSLOT
    NSLOT = NCORES * SLOTS_PC
    BSZ = NSLOT // 4
    assert BSZ <= 32767, f"int16 gather range exceeded: BSZ={BSZ}"
    NBLK = SLOTS_PC // P  # phase-A blocks of 128 slots

    # --- slot assignment ---
    slot_of_node = np.full(N, -1, np.int64)
    for c in range(NCORES):
        for w, wl in enumerate(windows[c]):
            base = c * SLOTS_PC + w * WSLOT
            slot_of_node[np.asarray(wl, np.int64)] = base + np.arange(len(wl))
    assert (slot_of_node >= 0).all()

    sslot = slot_of_node[src]
    dslot = slot_of_node[dst]
    ecore = (dslot // SLOTS_PC).astype(np.int64)
    ew = (dslot % SLOTS_PC) // WSLOT
    eb = (sslot // BSZ).astype(np.int64)
    edloc = (dslot % WSLOT).astype(np.int64)
    esidx = (sslot % BSZ).astype(np.int64)

    key = ((ecore * NW + ew) * 4 + eb)
    eorder = np.argsort(key, kind="stable")
    key_s = key[eorder]
    cnt = np.bincount(key_s, minlength=NCORES * NW * 4)
    assert cnt.max() <= BCAP, f"bucket cap violated: {cnt.max()}"
    starts = np.zeros(NCORES * NW * 4 + 1, np.int64)
    np.cumsum(cnt, out=starts[1:])
    esidx_s = esidx[eorder]
    edloc_s = edloc[eorder]

    # gidx: [C, NG, 4, CALL]; dloc: [C, NG, 4, GT, 128]
    gidx = np.zeros((NCORES, NG, 4, CALL), np.int16)
    dloc = np.full((NCORES, NG, 4, GT, P), FILL, np.float32)
    for c in range(NCORES):
        for w in range(NW):
            g, wl = divmod(w, GRP)
            for b in range(4):
                k = (c * NW + w) * 4 + b
                lo, hi = starts[k], starts[k + 1]
                n = hi - lo
                if n == 0:
                    continue
                off = wl * TPB * P
                gidx[c, g, b, off:off + n] = esidx_s[lo:hi].astype(np.int16)
                tt = (np.arange(n) // P) + wl * TPB
                pp = np.arange(n) % P
                dloc[c, g, b, tt, pp] = edloc_s[lo:hi].astype(np.float32)

    # wrap-16 + replicate to 128 partitions: [C, NG*4*128, CALL//16]
    g16 = gidx.reshape(NCORES, NG * 4, CALL // 16, 16).transpose(0, 1, 3, 2)
    g128 = np.tile(g16, (1, 1, 8, 1)).reshape(NCORES, NG * 4 * 128, CALL // 16)
    # dstl: [C, 128, NG * 64]  col = g*64 + b*GT + t
    dl = dloc.transpose(0, 4, 1, 2, 3).reshape(NCORES, P, NG * 4 * GT)
    dl = np.ascontiguousarray(dl).astype(BF16)

    node_of_slot = np.full(NSLOT, -1, np.int64)
    node_of_slot[slot_of_node] = np.arange(N)
    xT = np.zeros((NCORES, D, SLOTS_PC), np.float32)
    for c in range(NCORES):
        sl = node_of_slot[c * SLOTS_PC:(c + 1) * SLOTS_PC]
        valid = sl >= 0
        blk = np.zeros((SLOTS_PC, D), np.float32)
        blk[valid] = x[sl[valid]]
        xT[c] = blk.T

    meta = dict(NW=NW, NG=NG, NBLK=NBLK, SLOTS_PC=SLOTS_PC, NSLOT=NSLOT,
                BSZ=BSZ, N=N)
    percore = dict(xT=xT, gidx=g128, dstl=dl)
    return meta, percore, node_of_slot


def _build_program(meta):
    import concourse.bacc as bacc
    import concourse.tile as tile
    from concourse import mybir

    F32, BF, I16 = mybir.dt.float32, mybir.dt.bfloat16, mybir.dt.int16
    Alu = mybir.AluOpType
    Act = mybir.ActivationFunctionType
    AxL = mybir.AxisListType

    NW, NG, NBLK = meta["NW"], meta["NG"], meta["NBLK"]
    SLOTS_PC, NSLOT, BSZ = meta["SLOTS_PC"], meta["NSLOT"], meta["BSZ"]
    BPG = GRP * WSLOT // P  # phase-A blocks per window-group (4)

    nc = bacc.Bacc("TRN2", target_bir_lowering=False, debug=False,
                   num_devices=NCORES, num_swdge_queues=4)

    xT_d = nc.dram_tensor("xT", [D, SLOTS_PC], F32, kind="ExternalInput")
    gidx_d = nc.dram_tensor("gidx", [NG * 4 * 128, CALL // 16], I16,
                            kind="ExternalInput")
    dstl_d = nc.dram_tensor("dstl", [P, NG * 4 * GT], BF,
                            kind="ExternalInput")
    w1cat_d = nc.dram_tensor("w1cat", [D, 67], F32, kind="ExternalInput")
    w2cat_d = nc.dram_tensor("w2cat", [D, 67], BF, kind="ExternalInput")
    wd1_d = nc.dram_tensor("wd1rep", [D, 128], F32, kind="ExternalInput")
    wd2_d = nc.dram_tensor("wd2rep", [D, 128], BF, kind="ExternalInput")
    b1rep_d = nc.dram_tensor("b1rep", [WSLOT, D], BF, kind="ExternalInput")
    b2rep_d = nc.dram_tensor("b2rep", [WSLOT, D], F32, kind="ExternalInput")
    ident_d = nc.dram_tensor("ident", [WSLOT, WSLOT], BF, kind="ExternalInput")
    out_d = nc.dram_tensor("outS", [SLOTS_PC, D], F32, kind="ExternalOutput")

    shard = [nc.dram_tensor(f"shard{l}", [SLOTS_PC, 128], BF) for l in (1, 2)]
    tbl = [nc.dram_tensor(f"tbl{l}", [NSLOT, 128], BF, addr_space="Shared")
           for l in (1, 2)]

    with tile.TileContext(nc) as tc:
        import contextlib
        stack = contextlib.ExitStack()
        with stack:
            const = stack.enter_context(tc.tile_pool(name="const", bufs=1))
            small = stack.enter_context(tc.tile_pool(name="small", bufs=3))
            vp = stack.enter_context(tc.tile_pool(name="vp", bufs=4))
            mp = stack.enter_context(tc.tile_pool(name="mp", bufs=3))
            sc = stack.enter_context(tc.tile_pool(name="sc", bufs=3))
            ep = stack.enter_context(tc.tile_pool(name="ep", bufs=3))
            psA = stack.enter_context(tc.tile_pool(name="psA", bufs=2, space="PSUM"))
            psB = stack.enter_context(tc.tile_pool(name="psB", bufs=2, space="PSUM"))
            psC = stack.enter_context(tc.tile_pool(name="psC", bufs=2, space="PSUM"))

            # constants
            iota_i = const.tile([P, WSLOT], I16)
            nc.gpsimd.iota(iota_i[:], pattern=[[1, WSLOT]], base=0,
                           channel_multiplier=0)
            iota_b = const.tile([P, WSLOT], BF)
            nc.vector.tensor_copy(iota_b[:], iota_i[:])
            w1cat = const.tile([D, 67], F32)
            nc.sync.dma_start(w1cat[:], w1cat_d.ap()[:])
            w2cat = const.tile([D, 67], BF)
            nc.sync.dma_start(w2cat[:], w2cat_d.ap()[:])
            wd1 = const.tile([D, 128], F32)
            nc.sync.dma_start(wd1[:], wd1_d.ap()[:])
            wd2 = const.tile([D, 128], BF)
            nc.sync.dma_start(wd2[:], wd2_d.ap()[:])
            b1rep = const.tile([WSLOT, D], BF)
            nc.sync.dma_start(b1rep[:], b1rep_d.ap()[:])
            b2rep = const.tile([WSLOT, D], F32)
            nc.sync.dma_start(b2rep[:], b2rep_d.ap()[:])
            ident = const.tile([WSLOT, WSLOT], BF)
            nc.sync.dma_start(ident[:], ident_d.ap()[:])

            # resident across layers
            x2T = const.tile([D, SLOTS_PC], BF)
            adrep = const.tile([P, SLOTS_PC], BF)

            def phase_a_block(layer, blk):
                wcat = w1cat if layer == 0 else w2cat
                wdrep = wd1 if layer == 0 else wd2
                shard_l = shard[layer].ap()
                if layer == 0:
                    xtw = small.tile([D, P], F32, tag="xtw")
                    nc.sync.dma_start(xtw[:], xT_d.ap()[:, blk * P:(blk + 1) * P])
                    lhs = xtw[:]
                else:
                    lhs = x2T[:, blk * P:(blk + 1) * P]
                ps_c = psA.tile([P, 67], F32)
                nc.tensor.matmul(ps_c[:], lhsT=lhs, rhs=wcat[:],
                                 start=True, stop=True)
                ps_ad = psB.tile([P, 128], F32)
                nc.tensor.matmul(ps_ad[:], lhsT=wdrep[:], rhs=lhs,
                                 start=True, stop=True)
                tblrow = small.tile([P, 67], BF, tag="tblrow")
                nc.scalar.copy(tblrow[:], ps_c[:])
                nc.vector.memset(tblrow[:, 64:65], 1.0)
                # a_s_lo residual for extra precision
                nc.vector.tensor_tensor(out=tblrow[:, 66:67],
                                        in0=ps_c[:, 65:66],
                                        in1=tblrow[:, 65:66],
                                        op=Alu.subtract)
                nc.scalar.dma_start(shard_l[blk * P:(blk + 1) * P, 0:67],
                                    tblrow[:])
                nc.scalar.copy(adrep[:, blk * P:(blk + 1) * P], ps_ad[:])

            def all_gather(layer):
                nc.gpsimd.collective_compute(
                    "AllGather", mybir.AluOpType.bypass,
                    replica_groups=[list(range(NCORES))],
                    ins=[shard[layer].ap()[:, :]], outs=[tbl[layer].ap()[:, :]],
                )

            def edge_group(layer, g):
                tbl_l = tbl[layer].ap()
                dstl = sc.tile([P, 4 * GT], BF, tag="dstl")
                nc.sync.dma_start(
                    dstl[:],
                    dstl_d.ap()[:, g * 4 * GT:(g + 1) * 4 * GT])
                vslab = vp.tile([P, 4, GT, 128], BF, tag="vslab")
                for b in range(4):
                    idxt = small.tile([128, CALL // 16], I16, tag="idxt")
                    r0 = (g * 4 + b) * 128
                    nc.sync.dma_start(idxt[:], gidx_d.ap()[r0:r0 + 128, :])
                    nc.gpsimd.dma_gather(
                        out_ap=vslab[:, b, :, :],
                        in_ap=tbl_l[b * BSZ:(b + 1) * BSZ, :],
                        idxs_ap=idxt[:], num_idxs=CALL, num_idxs_reg=CALL,
                        elem_size=128, single_packet=False, queue_num=b)

                M = mp.tile([P, 4, GT, WSLOT], BF, tag="M")
                madc = sc.tile([P, GT, WSLOT], BF, tag="madc")
                adpec = sc.tile([P, 4 * GT], F32, tag="adpec")
                nc.vector.tensor_tensor(
                    out=M[:],
                    in0=iota_b[:, None, None, :]
                        .to_broadcast([P, 4, GT, WSLOT]),
                    in1=dstl[:].rearrange("p (b t) -> p b t", b=4)
                        [:, :, :, None].to_broadcast([P, 4, GT, WSLOT]),
                    op=Alu.is_equal)
                for b in range(4):
                    nc.vector.tensor_tensor(
                        out=madc[:],
                        in0=M[:, b],
                        in1=adrep[:, g * GRP * WSLOT:(g + 1) * GRP * WSLOT]
                            .rearrange("p (w d) -> p w d", w=GRP)
                            [:, :, None, :]
                            .to_broadcast([P, GRP, TPB, WSLOT]),
                        op=Alu.mult)
                    nc.vector.tensor_reduce(
                        out=adpec[:, b * GT:(b + 1) * GT], in_=madc[:],
                        axis=AxL.X, op=Alu.add)
                # per-edge scores [P, 4*GT]
                as_t = sc.tile([P, 4 * GT], F32, tag="as_t")
                nc.vector.tensor_tensor(out=as_t[:],
                                        in0=vslab[:, :, :, 65],
                                        in1=vslab[:, :, :, 66],
                                        op=Alu.add)
                s_t = sc.tile([P, 4 * GT], F32, tag="s_t")
                nc.vector.tensor_tensor(
                    out=s_t[:], in0=as_t[:], in1=adpec[:], op=Alu.add)
                lr = sc.tile([P, 4 * GT], F32, tag="lr")
                nc.vector.scalar_tensor_tensor(
                    out=lr[:], in0=s_t[:], scalar=NEG_SLOPE,
                    in1=s_t[:], op0=Alu.mult, op1=Alu.max)
                ex = sc.tile([P, 4 * GT], F32, tag="ex")
                nc.scalar.activation(ex[:], lr[:], Act.Exp)
                # Mex = M * ex (in place)
                nc.vector.tensor_tensor(
                    out=M[:], in0=M[:],
                    in1=ex[:].rearrange("p (b t) -> p b t", b=4)
                        [:, :, :, None].to_broadcast([P, 4, GT, WSLOT]),
                    op=Alu.mult)
                # aggregation per window into PSUM, collected into aggsb
                aggsb = ep.tile([WSLOT, GRP, 65], F32, tag="aggsb")
                for wl in range(GRP):
                    psagg = psC.tile([WSLOT, 65], F32)
                    k = 0
                    for b in range(4):
                        for t in range(TPB):
                            nc.tensor.matmul(
                                psagg[:],
                                lhsT=M[:, b, wl * TPB + t, :],
                                rhs=vslab[:, b, wl * TPB + t, 0:65],
                                start=(k == 0), stop=(k == 4 * TPB - 1))
                            k += 1
                    nc.scalar.copy(aggsb[:, wl, :], psagg[:])
                # batched epilogue over the group's GRP windows
                den = ep.tile([WSLOT, GRP], F32, tag="den")
                nc.vector.tensor_scalar_add(den[:], aggsb[:, :, 64], EPS)
                rec = ep.tile([WSLOT, GRP], F32, tag="rec")
                nc.vector.reciprocal(rec[:], den[:])
                if layer == 0:
                    t1 = ep.tile([WSLOT, GRP, D], BF, tag="t1")
                    nc.vector.tensor_tensor(
                        out=t1[:], in0=aggsb[:, :, 0:64],
                        in1=rec[:, :, None].to_broadcast([WSLOT, GRP, D]),
                        op=Alu.mult)
                    u = ep.tile([WSLOT, GRP, D], BF, tag="u")
                    nc.vector.tensor_tensor(
                        out=u[:], in0=t1[:],
                        in1=b1rep[:, None, :].to_broadcast([WSLOT, GRP, D]),
                        op=Alu.add)
                    for wl in range(GRP):
                        w = g * GRP + wl
                        ps_t = psB.tile([D, WSLOT], F32)
                        nc.tensor.matmul(ps_t[:], lhsT=u[:, wl, :],
                                         rhs=ident[:], start=True, stop=True)
                        nc.scalar.activation(
                            x2T[:, w * WSLOT:(w + 1) * WSLOT], ps_t[:],
                            Act.Relu)
                else:
                    t1 = ep.tile([WSLOT, GRP, D], F32, tag="t1f")
                    nc.vector.tensor_tensor(
                        out=t1[:], in0=aggsb[:, :, 0:64],
                        in1=rec[:, :, None].to_broadcast([WSLOT, GRP, D]),
                        op=Alu.mult)
                    u = ep.tile([WSLOT, GRP, D], F32, tag="uf")
                    nc.vector.tensor_tensor(
                        out=u[:], in0=t1[:],
                        in1=b2rep[:, None, :].to_broadcast([WSLOT, GRP, D]),
                        op=Alu.add)
                    nc.sync.dma_start(
                        out_d.ap()[g * GRP * WSLOT:(g + 1) * GRP * WSLOT, :]
                            .rearrange("(w s) f -> s w f", w=GRP),
                        u[:])

            # ---- layer 1 ----
            for blk in range(NBLK):
                phase_a_block(0, blk)
            all_gather(0)
            for g in range(NG):
                edge_group(0, g)
                # emit layer-2 phase A for the windows just produced
                for blk in range(g * BPG, (g + 1) * BPG):
                    phase_a_block(1, blk)
            all_gather(1)
            for g in range(NG):
                edge_group(1, g)

    nc.compile()
    return nc


def _host_params(W1, att_src1, att_dst1, b1, W2, att_src2, att_dst2, b2):
    def cat(W, a_s):
        w = np.zeros((D, 67), np.float32)
        w[:, 0:64] = W
        w[:, 65] = W @ np.asarray(a_s, np.float32)
        return w
    w1cat = cat(W1, att_src1)
    w2cat = cat(W2, att_src2).astype(BF16)
    wd1 = np.tile((W1 @ np.asarray(att_dst1, np.float32))[:, None],
                  (1, 128)).astype(np.float32)
    wd2 = np.tile((W2 @ np.asarray(att_dst2, np.float32))[:, None],
                  (1, 128)).astype(BF16)
    b1r = np.tile(np.asarray(b1, np.float32)[None, :], (WSLOT, 1)).astype(BF16)
    b2r = np.tile(np.asarray(b2, np.float32)[None, :], (WSLOT, 1)).astype(np.float32)
    ident = np.eye(WSLOT, dtype=BF16)
    return dict(w1cat=w1cat, w2cat=w2cat, wd1rep=wd1, wd2rep=wd2,
                b1rep=b1r, b2rep=b2r, ident=ident)


def _in_maps(meta, percore, params):
    maps = []
    for c in range(NCORES):
        m = {"xT": percore["xT"][c], "gidx": percore["gidx"][c],
             "dstl": percore["dstl"][c]}
        m.update(params)
        maps.append(m)
    return maps


def kernel(x, edge_index, W1, att_src1, att_dst1, b1, W2, att_src2,
           att_dst2, b2):
    from concourse.bass_utils import run_bass_kernel_spmd

    x = np.asarray(x, np.float32)
    edge_index = np.asarray(edge_index)
    W1 = np.asarray(W1, np.float32)
    W2 = np.asarray(W2, np.float32)

    ek = edge_index.tobytes()
    cached = _CACHE.get("pre")
    if cached is not None and cached[0] == ek and \
            np.array_equal(cached[1], x):
        _, _, meta, percore, node_of_slot = cached
    else:
        meta, percore, node_of_slot = _preprocess(x, edge_index)
        _CACHE["pre"] = (ek, x.copy(), meta, percore, node_of_slot)
    mk = tuple(sorted(meta.items()))
    cached = _CACHE.get("prog")
    if cached is not None and cached[0] == mk:
        nc = cached[1]
    else:
        nc = _build_program(meta)
        _CACHE["prog"] = (mk, nc)
    SLOTS_PC, N = meta["SLOTS_PC"], meta["N"]

    params = _host_params(W1, att_src1, att_dst1, b1, W2, att_src2,
                          att_dst2, b2)
    res = run_bass_kernel_spmd(nc, _in_maps(meta, percore, params),
                               list(range(NCORES)))

    out = np.empty((N, D), np.float32)
    for c in range(NCORES):
        blk = res.results[c]["outS"]  # [SLOTS_PC, 64]
        sl = node_of_slot[c * SLOTS_PC:(c + 1) * SLOTS_PC]
        valid = sl >= 0
        out[sl[valid]] = blk[valid]
    return out


# revision 17
# speedup vs baseline: 1.3016x; 1.0271x over previous
"""Two-layer single-head GAT on Trainium2 (8 NeuronCores, Bass/Tile).

Strategy (graph-parallel over dst nodes), v2:
  - Nodes relabeled into slots: 8 cores x NW windows x 64 slots. Nodes
    assigned to cores balanced by degree (serpentine over degree-sorted
    order), then packed into windows (<=64 nodes, per-src-bucket in-degree
    <= 256; bucket(src) = src_core//2).
  - Per layer each core builds its shard of a node table
    row[n] = [h(64) | 1.0 | as_hi | as_lo] bf16 (128-col rows for the
    256B-stride dma_gather), AllGathered so every core holds the full
    table in DRAM.
  - Edges live on the dst core, grouped by (window-group g of GRP=8
    windows, src bucket b). Per (g,b) one dma_gather of CALL=2048 rows
    (int16 idx within the bucket slice), issued on SWDGE queue b so the
    four buckets' Q7 descriptor generation overlaps across core pairs.
  - Per (g,b), batched DVE ops build M = onehot(dstl) [128e x 16t x 64d],
    Madc = M * adrep (scratch), adpec = rowsum(Madc) = a_d[dst_e]; scores
    s = as_hi+as_lo+adpec, ex = exp(lrelu(s)) (ACT); Mex = M * ex.
  - Aggregation per window: PSUM [64d x 65] += Mex_tile^T @ vslab_tile
    (lhsT = Mex [128e x 64d], rhs = raw gathered rows [128e x 65] =
    [h | 1]); epilogue divides by the ex-sum column (col 64), adds bias;
    layer 1 transposes back to [64f x 64s] via identity matmul + ACT relu
    copy into the resident x2T; layer 2 DMAs slot-major fp32 output.
Softmax max-subtraction is skipped (scores are O(10); exp stays in fp32
range; alpha is mathematically identical). Output unpermuted on host.
"""

import numpy as np
import ml_dtypes

BF16 = ml_dtypes.bfloat16

NCORES = 8
P = 128
D = 64
NEG_SLOPE = 0.2
EPS = 1e-16

WSLOT = 64        # dst slots per window
NODECAP = 64      # max nodes per window
TPB = 2           # 128-edge tiles per (window, bucket)
BCAP = TPB * 128  # per-bucket in-degree cap per window
GRP = 8           # windows per gather group
GT = GRP * TPB    # tiles per (group, bucket)
CALL = GT * 128   # idxs per dma_gather call (keep < ~2500: walrus 16-bit)
FILL = 999.0      # dstl filler (never matches iota 0..63)

_CACHE = {}


def _preprocess(x, edge_index):
    """Host-side partitioning/indexing. Returns per-core input arrays + meta."""
    N = x.shape[0]
    src = edge_index[0].astype(np.int64)
    dst = edge_index[1].astype(np.int64)

    deg = np.bincount(dst, minlength=N)

    # --- assign nodes to cores: serpentine over degree-sorted order ---
    order = np.argsort(-deg, kind="stable")
    core_of_node = np.empty(N, np.int32)
    pat = np.concatenate([np.arange(NCORES), np.arange(NCORES)[::-1]])
    core_of_node[order] = pat[np.arange(N) % (2 * NCORES)]

    bucket_of_node = core_of_node // 2  # 4 buckets of 2 cores each

    deg_b = np.zeros((N, 4), np.int64)
    for b in range(4):
        m = bucket_of_node[src] == b
        deg_b[:, b] = np.bincount(dst[m], minlength=N)

    # --- pack windows per core: <=NODECAP nodes, per-bucket deg <= BCAP.
    # Best-fit decreasing over K open bins (first-fit leaves ~8% slack:
    # one bucket hits the cap while the others sit at ~94%).
    windows = [[] for _ in range(NCORES)]
    K = 8
    for c in range(NCORES):
        nodes_c = order[core_of_node[order] == c]  # degree-sorted
        bl = []            # node lists of open bins
        bb = np.zeros((0, 4), np.int64)  # bucket loads of open bins
        bn = np.zeros(0, np.int64)       # node counts
        for n in nodes_c:
            db = deg_b[n]
            cand = bb + db
            fits = (cand <= BCAP).all(axis=1) & (bn < NODECAP)
            if fits.any():
                score = np.where(fits, cand.max(axis=1), -1)
                i = int(np.argmax(score))
                bl[i].append(n)
                bb[i] += db
                bn[i] += 1
            else:
                if len(bl) >= K:
                    j = int(np.argmax(bb.sum(axis=1) + 10000 * (bn >= NODECAP)))
                    windows[c].append(bl.pop(j))
                    bb = np.delete(bb, j, axis=0)
                    bn = np.delete(bn, j)
                bl.append([n])
                bb = np.vstack([bb, db[None, :]])
                bn = np.append(bn, 1)
        order_close = np.argsort(-bb.sum(axis=1))
        for j in order_close:
            windows[c].append(bl[j])

    nw_real = max(len(w) for w in windows)
    NG = -(-nw_real // GRP)
    NW = NG * GRP
    SLOTS_PC = NW * WSLOT
    NSLOT = NCORES * SLOTS_PC
    BSZ = NSLOT // 4
    assert BSZ <= 32767, f"int16 gather range exceeded: BSZ={BSZ}"
    NBLK = SLOTS_PC // P  # phase-A blocks of 128 slots

    # --- slot assignment ---
    slot_of_node = np.full(N, -1, np.int64)
    for c in range(NCORES):
        for w, wl in enumerate(windows[c]):
            base = c * SLOTS_PC + w * WSLOT
            slot_of_node[np.asarray(wl, np.int64)] = base + np.arange(len(wl))
    assert (slot_of_node >= 0).all()

    sslot = slot_of_node[src]
    dslot = slot_of_node[dst]
    ecore = (dslot // SLOTS_PC).astype(np.int64)
    ew = (dslot % SLOTS_PC) // WSLOT
    eb = (sslot // BSZ).astype(np.int64)
    edloc = (dslot % WSLOT).astype(np.int64)
    esidx = (sslot % BSZ).astype(np.int64)

    key = ((ecore * NW + ew) * 4 + eb)
    eorder = np.argsort(key, kind="stable")
    key_s = key[eorder]
    cnt = np.bincount(key_s, minlength=NCORES * NW * 4)
    assert cnt.max() <= BCAP, f"bucket cap violated: {cnt.max()}"
    starts = np.zeros(NCORES * NW * 4 + 1, np.int64)
    np.cumsum(cnt, out=starts[1:])
    esidx_s = esidx[eorder]
    edloc_s = edloc[eorder]

    # gidx: [C, NG, 4, CALL]; dloc: [C, NG, 4, GT, 128]
    gidx = np.zeros((NCORES, NG, 4, CALL), np.int16)
    dloc = np.full((NCORES, NG, 4, GT, P), FILL, np.float32)
    for c in range(NCORES):
        for w in range(NW):
            g, wl = divmod(w, GRP)
            for b in range(4):
                k = (c * NW + w) * 4 + b
                lo, hi = starts[k], starts[k + 1]
                n = hi - lo
                if n == 0:
                    continue
                off = wl * TPB * P
                gidx[c, g, b, off:off + n] = esidx_s[lo:hi].astype(np.int16)
                tt = (np.arange(n) // P) + wl * TPB
                pp = np.arange(n) % P
                dloc[c, g, b, tt, pp] = edloc_s[lo:hi].astype(np.float32)

    # wrap-16 + replicate to 128 partitions: [C, NG*4*128, CALL//16]
    g16 = gidx.reshape(NCORES, NG * 4, CALL // 16, 16).transpose(0, 1, 3, 2)
    g128 = np.tile(g16, (1, 1, 8, 1)).reshape(NCORES, NG * 4 * 128, CALL // 16)
    # dstl: [C, 128, NG * 64]  col = g*64 + b*GT + t
    dl = dloc.transpose(0, 4, 1, 2, 3).reshape(NCORES, P, NG * 4 * GT)
    dl = np.ascontiguousarray(dl).astype(BF16)

    node_of_slot = np.full(NSLOT, -1, np.int64)
    node_of_slot[slot_of_node] = np.arange(N)
    xT = np.zeros((NCORES, D, SLOTS_PC), np.float32)
    for c in range(NCORES):
        sl = node_of_slot[c * SLOTS_PC:(c + 1) * SLOTS_PC]
        valid = sl >= 0
        blk = np.zeros((SLOTS_PC, D), np.float32)
        blk[valid] = x[sl[valid]]
        xT[c] = blk.T

    meta = dict(NW=NW, NG=NG, NBLK=NBLK, SLOTS_PC=SLOTS_PC, NSLOT=NSLOT,
                BSZ=BSZ, N=N)
    percore = dict(xT=xT, gidx=g128, dstl=dl)
    return meta, percore, node_of_slot


def _build_program(meta):
    import concourse.bacc as bacc
    import concourse.tile as tile
    from concourse import mybir

    F32, BF, I16 = mybir.dt.float32, mybir.dt.bfloat16, mybir.dt.int16
    Alu = mybir.AluOpType
    Act = mybir.ActivationFunctionType
    AxL = mybir.AxisListType

    NW, NG, NBLK = meta["NW"], meta["NG"], meta["NBLK"]
    SLOTS_PC, NSLOT, BSZ = meta["SLOTS_PC"], meta["NSLOT"], meta["BSZ"]
    BPG = GRP * WSLOT // P  # phase-A blocks per window-group (4)

    nc = bacc.Bacc("TRN2", target_bir_lowering=False, debug=False,
                   num_devices=NCORES, num_swdge_queues=4)

    xT_d = nc.dram_tensor("xT", [D, SLOTS_PC], F32, kind="ExternalInput")
    gidx_d = nc.dram_tensor("gidx", [NG * 4 * 128, CALL // 16], I16,
                            kind="ExternalInput")
    dstl_d = nc.dram_tensor("dstl", [P, NG * 4 * GT], BF,
                            kind="ExternalInput")
    w1cat_d = nc.dram_tensor("w1cat", [D, 67], F32, kind="ExternalInput")
    w2cat_d = nc.dram_tensor("w2cat", [D, 67], BF, kind="ExternalInput")
    wd1_d = nc.dram_tensor("wd1rep", [D, 128], F32, kind="ExternalInput")
    wd2_d = nc.dram_tensor("wd2rep", [D, 128], BF, kind="ExternalInput")
    b1rep_d = nc.dram_tensor("b1rep", [WSLOT, D], BF, kind="ExternalInput")
    b2rep_d = nc.dram_tensor("b2rep", [WSLOT, D], F32, kind="ExternalInput")
    ident_d = nc.dram_tensor("ident", [WSLOT, WSLOT], BF, kind="ExternalInput")
    out_d = nc.dram_tensor("outS", [SLOTS_PC, D], F32, kind="ExternalOutput")

    shard = [nc.dram_tensor(f"shard{l}", [SLOTS_PC, 128], BF) for l in (1, 2)]
    tbl = [nc.dram_tensor(f"tbl{l}", [NSLOT, 128], BF, addr_space="Shared")
           for l in (1, 2)]

    with tile.TileContext(nc) as tc:
        import contextlib
        stack = contextlib.ExitStack()
        with stack:
            const = stack.enter_context(tc.tile_pool(name="const", bufs=1))
            small = stack.enter_context(tc.tile_pool(name="small", bufs=3))
            vp = stack.enter_context(tc.tile_pool(name="vp", bufs=4))
            mp = stack.enter_context(tc.tile_pool(name="mp", bufs=3))
            sc = stack.enter_context(tc.tile_pool(name="sc", bufs=3))
            ep = stack.enter_context(tc.tile_pool(name="ep", bufs=3))
            psA = stack.enter_context(tc.tile_pool(name="psA", bufs=2, space="PSUM"))
            psB = stack.enter_context(tc.tile_pool(name="psB", bufs=2, space="PSUM"))
            psC = stack.enter_context(tc.tile_pool(name="psC", bufs=2, space="PSUM"))

            # constants
            iota_i = const.tile([P, WSLOT], I16)
            nc.gpsimd.iota(iota_i[:], pattern=[[1, WSLOT]], base=0,
                           channel_multiplier=0)
            iota_b = const.tile([P, WSLOT], BF)
            nc.vector.tensor_copy(iota_b[:], iota_i[:])
            w1cat = const.tile([D, 67], F32)
            nc.sync.dma_start(w1cat[:], w1cat_d.ap()[:])
            w2cat = const.tile([D, 67], BF)
            nc.sync.dma_start(w2cat[:], w2cat_d.ap()[:])
            wd1 = const.tile([D, 128], F32)
            nc.sync.dma_start(wd1[:], wd1_d.ap()[:])
            wd2 = const.tile([D, 128], BF)
            nc.sync.dma_start(wd2[:], wd2_d.ap()[:])
            b1rep = const.tile([WSLOT, D], BF)
            nc.sync.dma_start(b1rep[:], b1rep_d.ap()[:])
            b2rep = const.tile([WSLOT, D], F32)
            nc.sync.dma_start(b2rep[:], b2rep_d.ap()[:])
            ident = const.tile([WSLOT, WSLOT], BF)
            nc.sync.dma_start(ident[:], ident_d.ap()[:])

            # resident across layers
            x2T = const.tile([D, SLOTS_PC], BF)
            adrep = const.tile([P, SLOTS_PC], BF)

            def phase_a_block(layer, blk):
                wcat = w1cat if layer == 0 else w2cat
                wdrep = wd1 if layer == 0 else wd2
                shard_l = shard[layer].ap()
                if layer == 0:
                    xtw = small.tile([D, P], F32, tag="xtw")
                    nc.sync.dma_start(xtw[:], xT_d.ap()[:, blk * P:(blk + 1) * P])
                    lhs = xtw[:]
                else:
                    lhs = x2T[:, blk * P:(blk + 1) * P]
                ps_c = psA.tile([P, 67], F32)
                nc.tensor.matmul(ps_c[:], lhsT=lhs, rhs=wcat[:],
                                 start=True, stop=True)
                ps_ad = psB.tile([P, 128], F32)
                nc.tensor.matmul(ps_ad[:], lhsT=wdrep[:], rhs=lhs,
                                 start=True, stop=True)
                tblrow = small.tile([P, 67], BF, tag="tblrow")
                nc.scalar.copy(tblrow[:], ps_c[:])
                nc.vector.memset(tblrow[:, 64:65], 1.0)
                # a_s_lo residual for extra precision
                nc.vector.tensor_tensor(out=tblrow[:, 66:67],
                                        in0=ps_c[:, 65:66],
                                        in1=tblrow[:, 65:66],
                                        op=Alu.subtract)
                nc.scalar.dma_start(shard_l[blk * P:(blk + 1) * P, 0:67],
                                    tblrow[:])
                nc.scalar.copy(adrep[:, blk * P:(blk + 1) * P], ps_ad[:])

            def all_gather(layer):
                nc.gpsimd.collective_compute(
                    "AllGather", mybir.AluOpType.bypass,
                    replica_groups=[list(range(NCORES))],
                    ins=[shard[layer].ap()[:, :]], outs=[tbl[layer].ap()[:, :]],
                )

            def edge_group(layer, g):
                tbl_l = tbl[layer].ap()
                dstl = sc.tile([P, 4 * GT], BF, tag="dstl")
                nc.sync.dma_start(
                    dstl[:],
                    dstl_d.ap()[:, g * 4 * GT:(g + 1) * 4 * GT])
                vslab = vp.tile([P, 4, GT, 128], BF, tag="vslab")
                for b in range(4):
                    idxt = small.tile([128, CALL // 16], I16, tag="idxt")
                    r0 = (g * 4 + b) * 128
                    nc.sync.dma_start(idxt[:], gidx_d.ap()[r0:r0 + 128, :])
                    nc.gpsimd.dma_gather(
                        out_ap=vslab[:, b, :, :],
                        in_ap=tbl_l[b * BSZ:(b + 1) * BSZ, :],
                        idxs_ap=idxt[:], num_idxs=CALL, num_idxs_reg=CALL,
                        elem_size=128, single_packet=False, queue_num=b)

                M = mp.tile([P, 4, GT, WSLOT], BF, tag="M")
                madc = sc.tile([P, GT, WSLOT], BF, tag="madc")
                adpec = sc.tile([P, 4 * GT], F32, tag="adpec")
                nc.vector.tensor_tensor(
                    out=M[:],
                    in0=iota_b[:, None, None, :]
                        .to_broadcast([P, 4, GT, WSLOT]),
                    in1=dstl[:].rearrange("p (b t) -> p b t", b=4)
                        [:, :, :, None].to_broadcast([P, 4, GT, WSLOT]),
                    op=Alu.is_equal)
                for b in range(4):
                    nc.vector.tensor_tensor(
                        out=madc[:],
                        in0=M[:, b],
                        in1=adrep[:, g * GRP * WSLOT:(g + 1) * GRP * WSLOT]
                            .rearrange("p (w d) -> p w d", w=GRP)
                            [:, :, None, :]
                            .to_broadcast([P, GRP, TPB, WSLOT]),
                        op=Alu.mult)
                    nc.vector.tensor_reduce(
                        out=adpec[:, b * GT:(b + 1) * GT], in_=madc[:],
                        axis=AxL.X, op=Alu.add)
                # per-edge scores [P, 4*GT]
                as_t = sc.tile([P, 4 * GT], F32, tag="as_t")
                nc.vector.tensor_tensor(out=as_t[:],
                                        in0=vslab[:, :, :, 65],
                                        in1=vslab[:, :, :, 66],
                                        op=Alu.add)
                s_t = sc.tile([P, 4 * GT], F32, tag="s_t")
                nc.vector.tensor_tensor(
                    out=s_t[:], in0=as_t[:], in1=adpec[:], op=Alu.add)
                lr = sc.tile([P, 4 * GT], F32, tag="lr")
                nc.vector.scalar_tensor_tensor(
                    out=lr[:], in0=s_t[:], scalar=NEG_SLOPE,
                    in1=s_t[:], op0=Alu.mult, op1=Alu.max)
                ex = sc.tile([P, 4 * GT], F32, tag="ex")
                nc.scalar.activation(ex[:], lr[:], Act.Exp)
                # Mex = M * ex (in place)
                nc.vector.tensor_tensor(
                    out=M[:], in0=M[:],
                    in1=ex[:].rearrange("p (b t) -> p b t", b=4)
                        [:, :, :, None].to_broadcast([P, 4, GT, WSLOT]),
                    op=Alu.mult)
                # aggregation per window into PSUM, collected into aggsb
                aggsb = ep.tile([WSLOT, GRP, 65], F32, tag="aggsb")
                for wl in range(GRP):
                    psagg = psC.tile([WSLOT, 65], F32)
                    k = 0
                    for b in range(4):
                        for t in range(TPB):
                            nc.tensor.matmul(
                                psagg[:],
                                lhsT=M[:, b, wl * TPB + t, :],
                                rhs=vslab[:, b, wl * TPB + t, 0:65],
                                start=(k == 0), stop=(k == 4 * TPB - 1))
                            k += 1
                    nc.scalar.copy(aggsb[:, wl, :], psagg[:])
                # batched epilogue over the group's GRP windows
                den = ep.tile([WSLOT, GRP], F32, tag="den")
                nc.vector.tensor_scalar_add(den[:], aggsb[:, :, 64], EPS)
                rec = ep.tile([WSLOT, GRP], F32, tag="rec")
                nc.vector.reciprocal(rec[:], den[:])
                if layer == 0:
                    t1 = ep.tile([WSLOT, GRP, D], BF, tag="t1")
                    nc.vector.tensor_tensor(
                        out=t1[:], in0=aggsb[:, :, 0:64],
                        in1=rec[:, :, None].to_broadcast([WSLOT, GRP, D]),
                        op=Alu.mult)
                    u = ep.tile([WSLOT, GRP, D], BF, tag="u")
                    nc.vector.tensor_tensor(
                        out=u[:], in0=t1[:],
                        in1=b1rep[:, None, :].to_broadcast([WSLOT, GRP, D]),
                        op=Alu.add)
                    for wl in range(GRP):
                        w = g * GRP + wl
                        ps_t = psB.tile([D, WSLOT], F32)
                        nc.tensor.matmul(ps_t[:], lhsT=u[:, wl, :],
                                         rhs=ident[:], start=True, stop=True)
                        nc.scalar.activation(
                            x2T[:, w * WSLOT:(w + 1) * WSLOT], ps_t[:],
                            Act.Relu)
                else:
                    t1 = ep.tile([WSLOT, GRP, D], F32, tag="t1f")
                    nc.vector.tensor_tensor(
                        out=t1[:], in0=aggsb[:, :, 0:64],
                        in1=rec[:, :, None].to_broadcast([WSLOT, GRP, D]),
                        op=Alu.mult)
                    u = ep.tile([WSLOT, GRP, D], F32, tag="uf")
                    nc.vector.tensor_tensor(
                        out=u[:], in0=t1[:],
                        in1=b2rep[:, None, :].to_broadcast([WSLOT, GRP, D]),
                        op=Alu.add)
                    nc.sync.dma_start(
                        out_d.ap()[g * GRP * WSLOT:(g + 1) * GRP * WSLOT, :]
                            .rearrange("(w s) f -> s w f", w=GRP),
                        u[:])

            # ---- layer 1 ----
            for blk in range(NBLK):
                phase_a_block(0, blk)
            all_gather(0)
            for g in range(NG):
                edge_group(0, g)
                # emit layer-2 phase A for the windows just produced
                for blk in range(g * BPG, (g + 1) * BPG):
                    phase_a_block(1, blk)
            all_gather(1)
            for g in range(NG):
                edge_group(1, g)

    nc.compile()
    return nc


def _host_params(W1, att_src1, att_dst1, b1, W2, att_src2, att_dst2, b2):
    def cat(W, a_s):
        w = np.zeros((D, 67), np.float32)
        w[:, 0:64] = W
        w[:, 65] = W @ np.asarray(a_s, np.float32)
        return w
    w1cat = cat(W1, att_src1)
    w2cat = cat(W2, att_src2).astype(BF16)
    wd1 = np.tile((W1 @ np.asarray(att_dst1, np.float32))[:, None],
                  (1, 128)).astype(np.float32)
    wd2 = np.tile((W2 @ np.asarray(att_dst2, np.float32))[:, None],
                  (1, 128)).astype(BF16)
    b1r = np.tile(np.asarray(b1, np.float32)[None, :], (WSLOT, 1)).astype(BF16)
    b2r = np.tile(np.asarray(b2, np.float32)[None, :], (WSLOT, 1)).astype(np.float32)
    ident = np.eye(WSLOT, dtype=BF16)
    return dict(w1cat=w1cat, w2cat=w2cat, wd1rep=wd1, wd2rep=wd2,
                b1rep=b1r, b2rep=b2r, ident=ident)


def _in_maps(meta, percore, params):
    maps = []
    for c in range(NCORES):
        m = {"xT": percore["xT"][c], "gidx": percore["gidx"][c],
             "dstl": percore["dstl"][c]}
        m.update(params)
        maps.append(m)
    return maps


def kernel(x, edge_index, W1, att_src1, att_dst1, b1, W2, att_src2,
           att_dst2, b2):
    from concourse.bass_utils import run_bass_kernel_spmd

    x = np.asarray(x, np.float32)
    edge_index = np.asarray(edge_index)
    W1 = np.asarray(W1, np.float32)
    W2 = np.asarray(W2, np.float32)

    ek = edge_index.tobytes()
    cached = _CACHE.get("pre")
    if cached is not None and cached[0] == ek and \
            np.array_equal(cached[1], x):
        _, _, meta, percore, node_of_slot = cached
    else:
        meta, percore, node_of_slot = _preprocess(x, edge_index)
        _CACHE["pre"] = (ek, x.copy(), meta, percore, node_of_slot)
    mk = tuple(sorted(meta.items()))
    cached = _CACHE.get("prog")
    if cached is not None and cached[0] == mk:
        nc = cached[1]
    else:
        nc = _build_program(meta)
        _CACHE["prog"] = (mk, nc)
    SLOTS_PC, N = meta["SLOTS_PC"], meta["N"]

    params = _host_params(W1, att_src1, att_dst1, b1, W2, att_src2,
                          att_dst2, b2)
    res = run_bass_kernel_spmd(nc, _in_maps(meta, percore, params),
                               list(range(NCORES)))

    out = np.empty((N, D), np.float32)
    for c in range(NCORES):
        blk = res.results[c]["outS"]  # [SLOTS_PC, 64]
        sl = node_of_slot[c * SLOTS_PC:(c + 1) * SLOTS_PC]
        valid = sl >= 0
        out[sl[valid]] = blk[valid]
    return out
